# revision 1
# baseline (speedup 1.0000x reference)
"""Trainium2 Bass kernel for the moe_routing problem (nn_DAWN_69904887709893).

Token-parallel across 8 NeuronCores (256 tokens/core), neuron pools replicated.
Heavy einsums (feature/restore/attention) run in fp32r on the PE; every matmul
feeding router logits (W_all, logit projections, W_fk/W_rk, W_o) runs in plain
fp32 so top-k selections match the fp32 reference. K^T and token-major V are
exchanged with a 4-rank AllGather per sequence group; causality comes from a
host-provided additive mask so the SPMD program is identical on every core.

PSUM accumulators that pack two regions per bank are pre-zeroed with memset and
use start=False matmuls throughout: correct regardless of has_written state and
immune to instruction reordering (a start=True clears the whole bank's bits).
"""
import sys

sys.path.insert(0, "/opt/trn_rl_repo")
import numpy as np
import concourse.bass as bass
import concourse.bacc as bacc
import concourse.mybir as mybir
import concourse.tile as tile
from concourse.bass_utils import run_bass_kernel_spmd
from concourse.masks import make_identity

F32 = mybir.dt.float32
F32R = mybir.dt.float32r
AX = mybir.AxisListType.X
OP = mybir.AluOpType
ACT = mybir.ActivationFunctionType

NCORES = 8
B, S, D, R, N, DS, TOPK, H = 2, 1024, 1024, 512, 32, 64, 4, 16
T = B * S
TL = T // NCORES          # tokens per core (256)
MT = TL // 128            # token tiles per core (2)
KT = D // 128             # contraction tiles over D (8)
DH = D // H               # head dim (64)
NRT = (N * R) // 128      # contraction tiles over N*R (128)
SEQ_BLOCKS = S // 128     # k blocks per sequence (8)
NEG = -1.0e30

# emb segment used by each of the 6 attention routings (fq, fk, fv, rq, rk, rv)
ATTN_SEG = [0, 0, 1, 2, 2, 3]

_PROG = None


def _routing(nc, rt, psmm, name, lhsT_ap, e_ap, wout):
    """softmax over 32 logits + top-4 sparsify + renormalize -> wout [128,32] f32.
    lhsT_ap: [64, 128] fp32 (h-segment transposed), e_ap: [64, 32] fp32."""
    lg = psmm.tile([128, N], F32, name=f"lg_{name}", tag="mm")
    nc.tensor.matmul(lg[:], lhsT_ap, e_ap, start=True, stop=True)
    mx = rt.tile([128, 1], F32, name=f"mx_{name}", tag="mx")
    nc.vector.tensor_reduce(mx[:], lg[:], AX, OP.max)
    nmx = rt.tile([128, 1], F32, name=f"nmx_{name}", tag="nmx")
    nc.scalar.mul(nmx[:], mx[:], -1.0)
    ex = rt.tile([128, N], F32, name=f"ex_{name}", tag="ex")
    ssum = rt.tile([128, 1], F32, name=f"ssum_{name}", tag="ssum")
    nc.scalar.activation(ex[:], lg[:], ACT.Exp, bias=nmx[:], scale=1.0,
                         accum_out=ssum[:])
    rs = rt.tile([128, 1], F32, name=f"rs_{name}", tag="rs")
    nc.vector.reciprocal(rs[:], ssum[:])
    sm = rt.tile([128, N], F32, name=f"sm_{name}", tag="sm")
    nc.vector.tensor_scalar_mul(sm[:], ex[:], rs[:])
    top8 = rt.tile([128, 8], F32, name=f"top8_{name}", tag="top8")
    nc.vector.max(top8[:], sm[:])
    ge = rt.tile([128, N], F32, name=f"ge_{name}", tag="ge")
    nc.vector.tensor_scalar(ge[:], sm[:], top8[:, 3:4], None, OP.is_ge)
    sp = rt.tile([128, N], F32, name=f"sp_{name}", tag="sp")
    nc.vector.tensor_mul(sp[:], sm[:], ge[:])
    s2 = rt.tile([128, 1], F32, name=f"s2_{name}", tag="s2")
    nc.vector.tensor_reduce(s2[:], sp[:], AX, OP.add)
    s2e = rt.tile([128, 1], F32, name=f"s2e_{name}", tag="s2e")
    nc.vector.tensor_scalar_add(s2e[:], s2[:], 1e-8)
    rs2 = rt.tile([128, 1], F32, name=f"rs2_{name}", tag="rs2")
    nc.vector.reciprocal(rs2[:], s2e[:])
    nc.vector.tensor_scalar_mul(wout, sp[:], rs2[:])


def _layernorm(nc, lnp, name, x_ap, s_bc, b_bc, out_ap):
    """LN over the free dim (D). x_ap/out_ap [128, D] f32; s_bc/b_bc [128, D]."""
    mu = lnp.tile([128, 1], F32, name=f"mu_{name}", tag="mu")
    nc.vector.tensor_reduce(mu[:], x_ap, AX, OP.add)
    nc.scalar.mul(mu[:], mu[:], 1.0 / D)
    xc = lnp.tile([128, D], F32, name=f"xc_{name}", tag="xc")
    nc.vector.tensor_scalar_sub(xc[:], x_ap, mu[:])
    sq = lnp.tile([128, D], F32, name=f"sq_{name}", tag="sq")
    vs = lnp.tile([128, 1], F32, name=f"vs_{name}", tag="vs")
    nc.scalar.activation(sq[:], xc[:], ACT.Square, accum_out=vs[:])
    nc.scalar.activation(vs[:], vs[:], ACT.Copy, scale=1.0 / D, bias=1e-6)
    rv = lnp.tile([128, 1], F32, name=f"rv_{name}", tag="rv")
    nc.vector.reciprocal(rv[:], vs[:])
    rstd = lnp.tile([128, 1], F32, name=f"rstd_{name}", tag="rstd")
    nc.scalar.activation(rstd[:], rv[:], ACT.Sqrt)
    nc.vector.scalar_tensor_tensor(out_ap, xc[:], rstd[:], s_bc, OP.mult, OP.mult)
    nc.vector.tensor_add(out_ap, out_ap, b_bc)


def _tr(nc, pstr, name, src_ap, ident, outs):
    """PE-transpose a [128, <=128] block; copy the psum into each (ap, engine)."""
    p = pstr.tile([src_ap.shape[-1], 128], src_ap.dtype, name=f"tr_{name}",
                  tag="tr")
    nc.tensor.transpose(p[:], src_ap, ident)
    for ap, eng in outs:
        if eng == "v":
            nc.vector.tensor_copy(ap, p[:, :ap.shape[-1]])
        else:
            nc.scalar.copy(ap, p[:, :ap.shape[-1]])


def _feature(nc, fp, psf, fdram, nxT_r, routes, engs):
    """h[m] accumulators += w[:,n] * (nx @ f_n) for all 32 neurons.
    routes: list of (w_tiles_per_m, hacc_per_m). F is streamed in half-neuron
    chunks (4 k-tiles each) to halve SBUF residency."""
    HKT = KT // 2
    for n in range(N):
        pfs = [psf.tile([128, R], F32, name=f"pf{m}", tag=f"pf{m}")
               for m in range(MT)]
        for half in range(2):
            fc = fp.tile([128, HKT * R], F32R, name="fc", tag="fc")
            engs[(2 * n + half) % len(engs)].dma_start(
                fc[:], fdram[:, n, half * HKT * R:(half + 1) * HKT * R])
            for m in range(MT):
                for kk in range(HKT):
                    k = half * HKT + kk
                    nc.tensor.matmul(pfs[m][:],
                                     nxT_r[:, k, m * 128:(m + 1) * 128],
                                     fc[:, kk * R:(kk + 1) * R],
                                     start=(k == 0), stop=(k == KT - 1))
        for m in range(MT):
            for wt, hacc in routes:
                w_ap = wt[m][:, n:n + 1]
                if n == 0:
                    nc.vector.tensor_scalar(hacc[m][:], pfs[m][:], w_ap, None,
                                            OP.mult)
                else:
                    nc.vector.scalar_tensor_tensor(hacc[m][:], pfs[m][:], w_ap,
                                                   hacc[m][:], OP.mult, OP.add)


def _restore(nc, pools, rdram, routes, psy, engs):
    """Transposed-output restores sharing one streamed r matrix.
    routes: list of (hT_tile, wt_stage_dram, yT_out_ap).
    Packs len(routes) [128, TL] accumulators per PSUM bank; all matmuls use
    start=False on memset-zeroed banks."""
    nr = len(routes)
    assert nr in (1, 2)
    per_bank = 2
    n_banks = (KT * nr + per_bank - 1) // per_bank
    pys = [psy.tile([128, per_bank, TL], F32, name=f"ry{i}", tag=f"ry{i}")
           for i in range(n_banks)]
    for py in pys:
        nc.vector.memset(py[:], 0.0)

    def acc_ap(ri, dt):
        flat = dt * nr + ri
        return pys[flat // per_bank][:, flat % per_bank, :]

    rp, wbp, gtp = pools["rchunk"], pools["wb"], pools["gt"]
    for kt in range(NRT):
        n, rb = kt // 4, kt % 4
        wbs = []
        for ri, (hT, wtd, _) in enumerate(routes):
            if rb == 0:
                wb = wbp.tile([128, TL], F32, name=f"wb{ri}", tag=f"wb{ri}")
                nc.scalar.dma_start(wb[:],
                                    wtd[n:n + 1, :].broadcast_to([128, TL]))
                pools[f"_wb{ri}"] = wb
            wbs.append(pools[f"_wb{ri}"])
        if kt % 2 == 0:
            rc = rp.tile([128, 2, D], F32R, name="rc", tag="rc")
            engs[(kt // 2) % len(engs)].dma_start(rc[:], rdram[:, kt:kt + 2, :])
            pools["_rc"] = rc
        rc = pools["_rc"]
        for ri, (hT, _, _) in enumerate(routes):
            gt = gtp.tile([128, TL], F32R, name=f"gt{ri}", tag=f"gt{ri}")
            nc.vector.tensor_mul(gt[:], hT[:, rb, :], wbs[ri][:])
            for dt in range(KT):
                nc.tensor.matmul(acc_ap(ri, dt),
                                 rc[:, kt % 2, dt * 128:(dt + 1) * 128],
                                 gt[:], start=False, stop=(kt == NRT - 1))
    for ri, (_, _, yT_out) in enumerate(routes):
        for dt in range(KT):
            eng = nc.scalar if (dt + ri) % 2 == 0 else nc.vector
            if eng is nc.scalar:
                nc.scalar.copy(yT_out[:, dt, :], acc_ap(ri, dt))
            else:
                nc.vector.tensor_copy(yT_out[:, dt, :], acc_ap(ri, dt))


def build(dbg=False):
    nc = bacc.Bacc("TRN2", target_bir_lowering=False, debug=False,
                   num_devices=NCORES)

    x_d = nc.dram_tensor("x", [TL, D], F32, kind="ExternalInput")
    maskT_d = nc.dram_tensor("maskT", [S, TL], F32, kind="ExternalInput")
    wall_d = nc.dram_tensor("wall", [128, KT, 6 * DS], F32, kind="ExternalInput")
    wo_d = nc.dram_tensor("wo", [128, KT, D], F32, kind="ExternalInput")
    wfk_d = nc.dram_tensor("wfk", [128, KT, DS], F32, kind="ExternalInput")
    wrk_d = nc.dram_tensor("wrk", [128, KT, DS], F32, kind="ExternalInput")
    et_d = nc.dram_tensor("et", [DS, 6 * N], F32, kind="ExternalInput")
    fqk_d = nc.dram_tensor("fqk", [128, N, KT * R], F32R, kind="ExternalInput")
    fv_d = nc.dram_tensor("fv", [128, N, KT * R], F32R, kind="ExternalInput")
    fkn_d = nc.dram_tensor("fkn", [128, N, KT * R], F32R, kind="ExternalInput")
    rqk_d = nc.dram_tensor("rqk", [128, NRT, D], F32R, kind="ExternalInput")
    rv_d = nc.dram_tensor("rv", [128, NRT, D], F32R, kind="ExternalInput")
    rkn_d = nc.dram_tensor("rkn", [128, NRT, D], F32R, kind="ExternalInput")
    ln_d = nc.dram_tensor("lnrows", [4, D], F32, kind="ExternalInput")
    bias_d = nc.dram_tensor("biasrow", [1, 8 * DS], F32, kind="ExternalInput")
    y_d = nc.dram_tensor("y", [TL, D], F32, kind="ExternalOutput")

    dbg_t = {}

    def dbg_tensor(name, shape):
        dbg_t[name] = nc.dram_tensor("dbg_" + name, shape, F32,
                                     kind="ExternalOutput")
        return dbg_t[name]

    with tile.TileContext(nc) as tc:
        with (
            tc.tile_pool(name="perm", bufs=1) as perm,
            tc.tile_pool(name="dramp", bufs=1, space="DRAM") as dramp,
            tc.tile_pool(name="lnp", bufs=1) as lnp,
            tc.tile_pool(name="rtp", bufs=2) as rtp,
        ):
            # staging + collective buffers
            wt_stage = {k: dramp.tile([N, TL], F32, name=f"wt_{k}")
                        for k in ("rq", "rk", "rv", "rkn")}
            cc_in = dramp.tile([128, 2 * KT * TL], F32R, name="cc_in")
            cc_out = dramp.tile([4 * 128, 2 * KT * TL], F32R, name="cc_out")

            ident = perm.tile([128, 128], F32)
            make_identity(nc, ident[:])
            ident_r = perm.tile([128, 128], F32R)
            nc.vector.tensor_copy(ident_r[:], ident[:])
            ones_f = perm.tile([128, 1], F32)
            nc.gpsimd.memset(ones_f[:], 1.0)
            ones_r = perm.tile([128, 1], F32R)
            nc.vector.tensor_copy(ones_r[:], ones_f[:])
            bias_bc = perm.tile([128, 8 * DS], F32)
            nc.sync.dma_start(bias_bc[:], bias_d[0:1, :].broadcast_to([128, 8 * DS]))
            et_sb = perm.tile([DS, 6 * N], F32)
            nc.sync.dma_start(et_sb[:], et_d[:])
            # copy at partition base 64 for routings whose h-segment sits in
            # the upper half of a transposed tile (matmul requires equal bases)
            et_hi = perm.tile([128, 6 * N], F32)
            nc.sync.dma_start(et_hi[DS:2 * DS, :], et_d[:])
            x_sb = perm.tile([128, MT, D], F32)
            for m in range(MT):
                nc.sync.dma_start(x_sb[:, m, :], x_d[m * 128:(m + 1) * 128, :])
            yT_q = perm.tile([128, KT, TL], F32R)

            # ============ stage 1: LN1 + routing + features + restores ========
            with (
                tc.tile_pool(name="st1", bufs=1) as st1,
                tc.tile_pool(name="fchunk", bufs=2) as fp,
                tc.tile_pool(name="rchunk", bufs=2) as rp,
                tc.tile_pool(name="wbp", bufs=2) as wbp,
                tc.tile_pool(name="gtp", bufs=3) as gtp,
            ):
                nxT_r = st1.tile([128, KT, TL], F32R)
                h_q = [st1.tile([128, R], F32, name=f"h_q{m}") for m in range(MT)]
                h_k = [st1.tile([128, R], F32, name=f"h_k{m}") for m in range(MT)]
                h_v = [st1.tile([128, R], F32, name=f"h_v{m}") for m in range(MT)]
                hT = {k: st1.tile([128, 4, TL], F32, name=f"hT_{k}")
                      for k in ("q", "k", "v")}
                w_feat = {p: [st1.tile([128, N], F32, name=f"w{p}_{m}")
                              for m in range(MT)] for p in range(3)}
                yT_k = st1.tile([128, KT, TL], F32R)
                yT_v = st1.tile([128, KT, TL], F32R)
                v_tok = st1.tile([128, MT, D], F32R)

                with (
                    tc.tile_pool(name="st1a", bufs=1) as st1a,
                    tc.tile_pool(name="ps_tr", bufs=2, space="PSUM") as pstr,
                    tc.tile_pool(name="ps_mm", bufs=2, space="PSUM") as psmm,
                    tc.tile_pool(name="ps_feat", bufs=2, space="PSUM") as psf,
                    tc.tile_pool(name="wallp", bufs=2) as wallp,
                ):
                    nxT = st1a.tile([128, KT, TL], F32)
                    nx = st1a.tile([128, MT, D], F32)
                    ln1_bc = st1a.tile([128, 2, D], F32)
                    for i in range(2):
                        nc.sync.dma_start(ln1_bc[:, i, :],
                                          ln_d[i:i + 1, :].broadcast_to([128, D]))
                    for m in range(MT):
                        _layernorm(nc, lnp, f"ln1_{m}", x_sb[:, m, :],
                                   ln1_bc[:, 0, :], ln1_bc[:, 1, :], nx[:, m, :])
                    if dbg:
                        td = dbg_tensor("nx", [TL, D])
                        for m in range(MT):
                            nc.sync.dma_start(td[m * 128:(m + 1) * 128, :],
                                              nx[:, m, :])
                    for m in range(MT):
                        for k in range(KT):
                            _tr(nc, pstr, f"nx_{m}_{k}",
                                nx[:, m, k * 128:(k + 1) * 128], ident[:],
                                [(nxT[:, k, m * 128:(m + 1) * 128], "v"),
                                 (nxT_r[:, k, m * 128:(m + 1) * 128], "s")])

                    hall = st1a.tile([128, MT, 6 * DS], F32)
                    for m in range(MT):
                        ph = psmm.tile([128, 6 * DS], F32, name="ph", tag="mm")
                        for k in range(KT):
                            wt_k = wallp.tile([128, 6 * DS], F32, name="wal",
                                              tag="wal")
                            nc.sync.dma_start(wt_k[:], wall_d[:, k, :])
                            nc.tensor.matmul(ph[:],
                                             nxT[:, k, m * 128:(m + 1) * 128],
                                             wt_k[:],
                                             start=(k == 0), stop=(k == KT - 1))
                        nc.vector.tensor_add(hall[:, m, :], ph[:],
                                             bias_bc[:, :6 * DS])
                    hallT = st1a.tile([128, 3, TL], F32)
                    for m in range(MT):
                        for i in range(3):
                            _tr(nc, pstr, f"ha_{m}_{i}",
                                hall[:, m, i * 128:(i + 1) * 128], ident[:],
                                [(hallT[:, i, m * 128:(m + 1) * 128], "v")])
                    w_rest = {}
                    for p in range(6):
                        seg = ATTN_SEG[p]
                        tiles = w_feat[p] if p < 3 else \
                            [st1a.tile([128, N], F32, name=f"w{p}_{m}")
                             for m in range(MT)]
                        if p >= 3:
                            w_rest[p] = tiles
                        for m in range(MT):
                            base, ti = (p % 2) * DS, p // 2
                            e_src = et_sb if base == 0 else et_hi
                            e_ap = e_src[base:base + DS,
                                         seg * N:(seg + 1) * N]
                            _routing(nc, rtp, psmm, f"r{p}_{m}",
                                     hallT[base:base + DS, ti,
                                           m * 128:(m + 1) * 128],
                                     e_ap, tiles[m][:])
                    if dbg:
                        nm6 = ["w_fq", "w_fk", "w_fv", "w_rq", "w_rk", "w_rv"]
                        for p in range(6):
                            td = dbg_tensor(nm6[p], [TL, N])
                            tiles = w_feat[p] if p < 3 else w_rest[p]
                            for m in range(MT):
                                nc.sync.dma_start(td[m * 128:(m + 1) * 128, :],
                                                  tiles[m][:])
                    for p, key in [(3, "rq"), (4, "rk"), (5, "rv")]:
                        wtt = st1a.tile([N, TL], F32, name=f"wtt{p}")
                        for m in range(MT):
                            _tr(nc, pstr, f"wt_{p}_{m}", w_rest[p][m][:],
                                ident[:], [(wtt[:, m * 128:(m + 1) * 128], "v")])
                        nc.sync.dma_start(wt_stage[key][:], wtt[:])

                    # features (qk shared for Q and K; v)
                    _feature(nc, fp, psf, fqk_d, nxT_r,
                             [(w_feat[0], h_q), (w_feat[1], h_k)],
                             [nc.sync, nc.gpsimd])
                    _feature(nc, fp, psf, fv_d, nxT_r,
                             [(w_feat[2], h_v)], [nc.sync, nc.gpsimd])
                    if dbg:
                        for nm, hh in [("h_q", h_q), ("h_k", h_k), ("h_v", h_v)]:
                            td = dbg_tensor(nm, [TL, R])
                            for m in range(MT):
                                nc.sync.dma_start(td[m * 128:(m + 1) * 128, :],
                                                  hh[m][:])
                    for nm, hh in [("q", h_q), ("k", h_k), ("v", h_v)]:
                        for m in range(MT):
                            for rb in range(4):
                                _tr(nc, pstr, f"h{nm}_{m}_{rb}",
                                    hh[m][:, rb * 128:(rb + 1) * 128], ident[:],
                                    [(hT[nm][:, rb, m * 128:(m + 1) * 128], "v")])

                # restores: Q+K fused (stream r_qk once), then V
                pools = {"rchunk": rp, "wb": wbp, "gt": gtp}
                with tc.tile_pool(name="ps_y", bufs=1, space="PSUM") as psy:
                    _restore(nc, pools, rqk_d,
                             [(hT["q"][:], wt_stage["rq"], yT_q[:]),
                              (hT["k"][:], wt_stage["rk"], yT_k[:])],
                             psy, [nc.sync, nc.gpsimd])
                with tc.tile_pool(name="ps_y2", bufs=1, space="PSUM") as psy:
                    _restore(nc, pools, rv_d,
                             [(hT["v"][:], wt_stage["rv"], yT_v[:])],
                             psy, [nc.sync, nc.gpsimd])
                if dbg:
                    for nm, yy in [("yT_q", yT_q), ("yT_k", yT_k),
                                   ("yT_v", yT_v)]:
                        td = dbg_tensor(nm, [D, TL])
                        for dt in range(KT):
                            nc.gpsimd.dma_start(td[dt * 128:(dt + 1) * 128, :],
                                                yy[:, dt, :])

                with tc.tile_pool(name="ps_tr2", bufs=2, space="PSUM") as pstr2:
                    for dt in range(KT):
                        for m in range(MT):
                            _tr(nc, pstr2, f"v_{dt}_{m}",
                                yT_v[:, dt, m * 128:(m + 1) * 128], ident_r[:],
                                [(v_tok[:, m, dt * 128:(dt + 1) * 128], "s")])
                nc.sync.dma_start(cc_in[:, :KT * TL],
                                  yT_k[:].rearrange("p k t -> p (k t)"))
                for m in range(MT):
                    nc.sync.dma_start(
                        cc_in[:, KT * TL + m * D:KT * TL + (m + 1) * D],
                        v_tok[:, m, :])
            nc.gpsimd.collective_compute(
                "AllGather", OP.bypass,
                ins=[cc_in[:]],
                outs=[cc_out[:]],
                replica_groups=[[0, 1, 2, 3], [4, 5, 6, 7]],
            )

            # ============ stage 2: attention + W_o ============
            late_cm = tc.tile_pool(name="late", bufs=1)
            late = late_cm.__enter__()
            x2 = late.tile([128, MT, D], F32)
            ot_sb = late.tile([128, KT, TL], F32)
            with (
                tc.tile_pool(name="st2", bufs=1) as st2,
                tc.tile_pool(name="attp", bufs=3) as att,
                tc.tile_pool(name="ps_att", bufs=2, space="PSUM") as psa,
                tc.tile_pool(name="ps_ot", bufs=4, space="PSUM") as psot,
            ):
                maskT_sb = st2.tile([128, SEQ_BLOCKS, TL], F32)
                for kb in range(SEQ_BLOCKS):
                    nc.sync.dma_start(maskT_sb[:, kb, :],
                                      maskT_d[kb * 128:(kb + 1) * 128, :])
                kt_all = st2.tile([128, 4, KT * TL], F32R)
                v_all = st2.tile([128, 4, MT * D], F32R)
                for ch in range(4):
                    nc.sync.dma_start(kt_all[:, ch, :],
                                      cc_out[ch * 128:(ch + 1) * 128, :KT * TL])
                    nc.sync.dma_start(v_all[:, ch, :],
                                      cc_out[ch * 128:(ch + 1) * 128, KT * TL:])
                for hp in range(KT):
                    for hh in range(2):
                        pot = psot.tile([DH, TL], F32, name="pot", tag="pot")
                        nc.vector.memset(pot[:], 0.0)
                        h_idx = hp * 2 + hh
                        qt_ap = yT_q[hh * DH:(hh + 1) * DH, hp, :]
                        pss = psa.tile([1, TL], F32, name="pss", tag="pss")
                        nc.vector.memset(pss[:], 0.0)
                        for kb in range(SEQ_BLOCKS):
                            ch, m2 = kb // 2, kb % 2
                            ktap = kt_all[hh * DH:(hh + 1) * DH, ch,
                                          hp * TL + m2 * 128:
                                          hp * TL + (m2 + 1) * 128]
                            vap = v_all[:, ch,
                                        m2 * D + h_idx * DH:
                                        m2 * D + (h_idx + 1) * DH]
                            pscore = psa.tile([128, TL], F32, name="pscore",
                                              tag="pscore")
                            nc.tensor.matmul(pscore[:], ktap, qt_ap,
                                             start=True, stop=True)
                            msc = att.tile([128, TL], F32, name="msc", tag="msc")
                            nc.vector.tensor_add(msc[:], pscore[:],
                                                 maskT_sb[:, kb, :])
                            expt = att.tile([128, TL], F32R, name="expt",
                                            tag="expt")
                            nc.scalar.activation(expt[:], msc[:], ACT.Exp,
                                                 scale=0.125)
                            nc.tensor.matmul(pss[:], ones_r[:], expt[:],
                                             start=False,
                                             stop=(kb == SEQ_BLOCKS - 1))
                            nc.tensor.matmul(pot[:], vap, expt[:], start=False,
                                             stop=(kb == SEQ_BLOCKS - 1))
                        rsr = att.tile([1, TL], F32, name="rsr", tag="rsr")
                        nc.vector.reciprocal(rsr[:], pss[:])
                        rbc = att.tile([DH, TL], F32, name="rbc", tag="rbc")
                        nc.gpsimd.partition_broadcast(rbc[:], rsr[:], channels=DH)
                        otn = att.tile([DH, TL], F32, name="otn", tag="otn")
                        nc.vector.tensor_mul(otn[:], pot[:], rbc[:])
                        # SBUF->SBUF DMA can shift partitions (DVE cannot)
                        nc.sync.dma_start(ot_sb[hh * DH:(hh + 1) * DH, hp, :],
                                          otn[:])
                if dbg:
                    td = dbg_tensor("oT", [D, TL])
                    for dt in range(KT):
                        nc.sync.dma_start(td[dt * 128:(dt + 1) * 128, :],
                                          ot_sb[:, dt, :])

            with (
                tc.tile_pool(name="wop", bufs=3) as wop,
                tc.tile_pool(name="ps_mm2", bufs=2, space="PSUM") as psmm2,
            ):
                for blk in range(2):
                    wo_t = []
                    for k in range(KT):
                        wt_k = wop.tile([128, 512], F32, name=f"wo{k}", tag="wo")
                        nc.sync.dma_start(wt_k[:],
                                          wo_d[:, k, blk * 512:(blk + 1) * 512])
                        wo_t.append(wt_k)
                    for m in range(MT):
                        px = psmm2.tile([128, 512], F32, name="px", tag="mm")
                        for k in range(KT):
                            nc.tensor.matmul(px[:],
                                             ot_sb[:, k, m * 128:(m + 1) * 128],
                                             wo_t[k][:],
                                             start=(k == 0), stop=(k == KT - 1))
                        nc.vector.tensor_add(
                            x2[:, m, blk * 512:(blk + 1) * 512], px[:],
                            x_sb[:, m, blk * 512:(blk + 1) * 512])
            if dbg:
                td = dbg_tensor("x2", [TL, D])
                for m in range(MT):
                    nc.sync.dma_start(td[m * 128:(m + 1) * 128, :], x2[:, m, :])

            # ============ stage 3: knowledge circuit ============
            with (
                tc.tile_pool(name="st3", bufs=1) as st3,
                tc.tile_pool(name="fchunk2", bufs=2) as fp2,
                tc.tile_pool(name="rchunk2", bufs=2) as rp2,
                tc.tile_pool(name="wbp2", bufs=2) as wbp2,
                tc.tile_pool(name="gtp2", bufs=3) as gtp2,
            ):
                nx2T_r = st3.tile([128, KT, TL], F32R)
                h_kn = [st3.tile([128, R], F32, name=f"h_kn{m}")
                        for m in range(MT)]
                hT_kn = st3.tile([128, 4, TL], F32)
                w_kn = {}
                yT_kn = st3.tile([128, KT, TL], F32)
                with (
                    tc.tile_pool(name="st3a", bufs=1) as st3a,
                    tc.tile_pool(name="ps_tr3", bufs=2, space="PSUM") as pstr3,
                    tc.tile_pool(name="ps_mm3", bufs=2, space="PSUM") as psmm3,
                    tc.tile_pool(name="ps_feat3", bufs=2, space="PSUM") as psf3,
                ):
                    nx2 = st3a.tile([128, MT, D], F32)
                    ln2_bc = st3a.tile([128, 2, D], F32)
                    for i in range(2):
                        nc.sync.dma_start(
                            ln2_bc[:, i, :],
                            ln_d[i + 2:i + 3, :].broadcast_to([128, D]))
                    for m in range(MT):
                        _layernorm(nc, lnp, f"ln2_{m}", x2[:, m, :],
                                   ln2_bc[:, 0, :], ln2_bc[:, 1, :], nx2[:, m, :])
                    nx2T = st3a.tile([128, KT, TL], F32)
                    for m in range(MT):
                        for k in range(KT):
                            _tr(nc, pstr3, f"nx2_{m}_{k}",
                                nx2[:, m, k * 128:(k + 1) * 128], ident[:],
                                [(nx2T[:, k, m * 128:(m + 1) * 128], "v"),
                                 (nx2T_r[:, k, m * 128:(m + 1) * 128], "s")])
                    wk_sb = st3a.tile([128, KT, 2 * DS], F32)
                    nc.sync.dma_start(wk_sb[:, :, :DS], wfk_d[:])
                    nc.sync.dma_start(wk_sb[:, :, DS:], wrk_d[:])
                    hkT = st3a.tile([DS, 2, TL], F32)
                    for m in range(MT):
                        for j in range(2):
                            pk = psmm3.tile([128, DS], F32, name="pk", tag="mm")
                            for k in range(KT):
                                nc.tensor.matmul(
                                    pk[:], nx2T[:, k, m * 128:(m + 1) * 128],
                                    wk_sb[:, k, j * DS:(j + 1) * DS],
                                    start=(k == 0), stop=(k == KT - 1))
                            hk = rtp.tile([128, DS], F32, name=f"hk{m}{j}",
                                          tag="hk")
                            nc.vector.tensor_add(
                                hk[:], pk[:],
                                bias_bc[:, (6 + j) * DS:(7 + j) * DS])
                            _tr(nc, pstr3, f"hk_{m}_{j}", hk[:], ident[:],
                                [(hkT[:, j, m * 128:(m + 1) * 128], "v")])
                    for j, nm in [(0, "fkn"), (1, "rkn")]:
                        w_kn[nm] = []
                        for m in range(MT):
                            wt = st3.tile([128, N], F32, name=f"wkn{j}_{m}")
                            _routing(nc, rtp, psmm3, f"rk{j}_{m}",
                                     hkT[:, j, m * 128:(m + 1) * 128],
                                     et_sb[:, (4 + j) * N:(5 + j) * N], wt[:])
                            w_kn[nm].append(wt)
                    if dbg:
                        for nm, key in [("w_fknow", "fkn"), ("w_rknow", "rkn")]:
                            td = dbg_tensor(nm, [TL, N])
                            for m in range(MT):
                                nc.sync.dma_start(td[m * 128:(m + 1) * 128, :],
                                                  w_kn[key][m][:])
                    wtt = st3a.tile([N, TL], F32, name="wtt_kn")
                    for m in range(MT):
                        _tr(nc, pstr3, f"wt_kn_{m}", w_kn["rkn"][m][:],
                            ident[:], [(wtt[:, m * 128:(m + 1) * 128], "v")])
                    nc.sync.dma_start(wt_stage["rkn"][:], wtt[:])

                    _feature(nc, fp2, psf3, fkn_d, nx2T_r,
                             [(w_kn["fkn"], h_kn)], [nc.sync, nc.gpsimd])
                    if dbg:
                        td = dbg_tensor("h_know", [TL, R])
                        for m in range(MT):
                            nc.sync.dma_start(td[m * 128:(m + 1) * 128, :],
                                              h_kn[m][:])
                    for m in range(MT):
                        for rb in range(4):
                            _tr(nc, pstr3, f"hkn_{m}_{rb}",
                                h_kn[m][:, rb * 128:(rb + 1) * 128], ident[:],
                                [(hT_kn[:, rb, m * 128:(m + 1) * 128], "v")])

                pools3 = {"rchunk": rp2, "wb": wbp2, "gt": gtp2}
                with tc.tile_pool(name="ps_y3", bufs=1, space="PSUM") as psy3:
                    _restore(nc, pools3, rkn_d,
                             [(hT_kn[:], wt_stage["rkn"], yT_kn[:])],
                             psy3, [nc.sync, nc.gpsimd])

                out_sb = st3.tile([128, MT, D], F32)
                with tc.tile_pool(name="ps_fin", bufs=2, space="PSUM") as psfin:
                    for dt in range(KT):
                        for m in range(MT):
                            p = psfin.tile([128, 128], F32,
                                           name=f"fin_{dt}_{m}", tag="fin")
                            nc.tensor.transpose(
                                p[:], yT_kn[:, dt, m * 128:(m + 1) * 128],
                                ident[:])
                            nc.vector.tensor_add(
                                out_sb[:, m, dt * 128:(dt + 1) * 128], p[:],
                                x2[:, m, dt * 128:(dt + 1) * 128])
                for m in range(MT):
                    nc.sync.dma_start(y_d[m * 128:(m + 1) * 128, :],
                                      out_sb[:, m, :])
            late_cm.__exit__(None, None, None)

    nc.compile()
    return nc, dbg_t


def prep_inputs(inputs):
    f32 = np.float32
    x = np.ascontiguousarray(np.asarray(inputs["x"], f32).reshape(T, D))
    ne = np.asarray(inputs["neuron_emb"], f32)
    emb = ne / (np.linalg.norm(ne, axis=-1, keepdims=True) + 1e-8)

    def f_layout(f):
        f = np.asarray(f, f32)
        return np.ascontiguousarray(
            f.reshape(N, KT, 128, R).transpose(2, 0, 1, 3).reshape(128, N, KT * R))

    def r_layout(r):
        r = np.asarray(r, f32).reshape(N * R, D)
        return np.ascontiguousarray(r.reshape(NRT, 128, D).transpose(1, 0, 2))

    def w_layout(w):
        w = np.asarray(w, f32)
        return np.ascontiguousarray(
            w.reshape(KT, 128, w.shape[-1]).transpose(1, 0, 2))

    shared = {
        "wall": w_layout(inputs["W_all"]),
        "wo": w_layout(inputs["W_o"]),
        "wfk": w_layout(inputs["W_fk"]),
        "wrk": w_layout(inputs["W_rk"]),
        "et": np.ascontiguousarray(emb.T),
        "fqk": f_layout(inputs["f_qk"]),
        "fv": f_layout(inputs["f_v"]),
        "fkn": f_layout(inputs["f_know"]),
        "rqk": r_layout(inputs["r_qk"]),
        "rv": r_layout(inputs["r_v"]),
        "rkn": r_layout(inputs["r_know"]),
        "lnrows": np.ascontiguousarray(
            np.stack([np.asarray(inputs[k], f32)
                      for k in ("ln1_s", "ln1_b", "ln2_s", "ln2_b")])),
        "biasrow": np.ascontiguousarray(
            np.concatenate([np.asarray(inputs["b_all"], f32),
                            np.asarray(inputs["b_fk"], f32),
                            np.asarray(inputs["b_rk"], f32)])[None, :]),
    }
    per_core = []
    k_idx = np.arange(S)[:, None]
    for c in range(NCORES):
        ci = c % (S // TL)
        q_idx = ci * TL + np.arange(TL)[None, :]
        maskT = np.where(k_idx <= q_idx, 0.0, NEG).astype(f32)
        per_core.append({
            "x": np.ascontiguousarray(x[c * TL:(c + 1) * TL]),
            "maskT": np.ascontiguousarray(maskT),
            **shared,
        })
    return per_core


def kernel(**inputs):
    global _PROG
    if _PROG is None:
        _PROG = build(dbg=False)
    nc, _ = _PROG
    per_core = prep_inputs(inputs)
    res = run_bass_kernel_spmd(nc, per_core, core_ids=list(range(NCORES)))
    y = np.concatenate([res.results[c]["y"] for c in range(NCORES)], axis=0)
    return y.reshape(B, S, D).astype(np.float32)



# revision 15
# speedup vs baseline: 1.4234x; 1.4234x over previous
"""Trainium2 Bass kernel for the moe_routing problem (nn_DAWN_69904887709893).

Token-parallel across 8 NeuronCores (256 tokens/core), neuron pools replicated.
Heavy einsums (feature/restore/attention) run in fp32r on the PE; every matmul
feeding router logits (W_all, logit projections, W_fk/W_rk, W_o) runs in plain
fp32 so top-k selections match the fp32 reference. K^T and token-major V are
exchanged with a 4-rank AllGather per sequence group; causality comes from a
host-provided additive mask so the SPMD program is identical on every core.

PSUM accumulators that pack two regions per bank are pre-zeroed with memset and
use start=False matmuls throughout: correct regardless of has_written state and
immune to instruction reordering (a start=True clears the whole bank's bits).
"""
import sys

sys.path.insert(0, "/opt/trn_rl_repo")
import numpy as np
import concourse.bass as bass
import concourse.bacc as bacc
import concourse.mybir as mybir
import concourse.tile as tile
from concourse.bass_utils import run_bass_kernel_spmd
from concourse.masks import make_identity

F32 = mybir.dt.float32
F32R = mybir.dt.float32r
BF16 = mybir.dt.bfloat16
AX = mybir.AxisListType.X
OP = mybir.AluOpType
ACT = mybir.ActivationFunctionType

NCORES = 8
B, S, D, R, N, DS, TOPK, H = 2, 1024, 1024, 512, 32, 64, 4, 16
T = B * S
TL = T // NCORES          # tokens per core (256)
MT = TL // 128            # token tiles per core (2)
KT = D // 128             # contraction tiles over D (8)
DH = D // H               # head dim (64)
NRT = (N * R) // 128      # contraction tiles over N*R (128)
SEQ_BLOCKS = S // 128     # k blocks per sequence (8)
NEG = -1.0e30

# emb segment used by each of the 6 attention routings (fq, fk, fv, rq, rk, rv)
ATTN_SEG = [0, 0, 1, 2, 2, 3]

_PROG = None


def _routing(nc, rt, psmm, name, lhsT_ap, e_ap, wout):
    """softmax over 32 logits + top-4 sparsify + renormalize -> wout [128,32] f32.
    lhsT_ap: [64, 128] fp32 (h-segment transposed), e_ap: [64, 32] fp32."""
    lg = psmm.tile([128, N], F32, name=f"lg_{name}", tag="mm")
    nc.tensor.matmul(lg[:], lhsT_ap, e_ap, start=True, stop=True)
    mx = rt.tile([128, 1], F32, name=f"mx_{name}", tag="mx")
    nc.vector.tensor_reduce(mx[:], lg[:], AX, OP.max)
    nmx = rt.tile([128, 1], F32, name=f"nmx_{name}", tag="nmx")
    nc.scalar.mul(nmx[:], mx[:], -1.0)
    ex = rt.tile([128, N], F32, name=f"ex_{name}", tag="ex")
    ssum = rt.tile([128, 1], F32, name=f"ssum_{name}", tag="ssum")
    nc.scalar.activation(ex[:], lg[:], ACT.Exp, bias=nmx[:], scale=1.0,
                         accum_out=ssum[:])
    rs = rt.tile([128, 1], F32, name=f"rs_{name}", tag="rs")
    nc.vector.reciprocal(rs[:], ssum[:])
    sm = rt.tile([128, N], F32, name=f"sm_{name}", tag="sm")
    nc.vector.tensor_scalar_mul(sm[:], ex[:], rs[:])
    top8 = rt.tile([128, 8], F32, name=f"top8_{name}", tag="top8")
    nc.vector.max(top8[:], sm[:])
    ge = rt.tile([128, N], F32, name=f"ge_{name}", tag="ge")
    nc.vector.tensor_scalar(ge[:], sm[:], top8[:, 3:4], None, OP.is_ge)
    sp = rt.tile([128, N], F32, name=f"sp_{name}", tag="sp")
    nc.vector.tensor_mul(sp[:], sm[:], ge[:])
    s2 = rt.tile([128, 1], F32, name=f"s2_{name}", tag="s2")
    nc.vector.tensor_reduce(s2[:], sp[:], AX, OP.add)
    s2e = rt.tile([128, 1], F32, name=f"s2e_{name}", tag="s2e")
    nc.vector.tensor_scalar_add(s2e[:], s2[:], 1e-8)
    rs2 = rt.tile([128, 1], F32, name=f"rs2_{name}", tag="rs2")
    nc.vector.reciprocal(rs2[:], s2e[:])
    nc.vector.tensor_scalar_mul(wout, sp[:], rs2[:])


def _layernorm(nc, lnp, name, x_ap, s_bc, b_bc, out_ap):
    """LN over the free dim (D). x_ap/out_ap [128, D] f32; s_bc/b_bc [128, D]."""
    mu = lnp.tile([128, 1], F32, name=f"mu_{name}", tag="mu")
    nc.vector.tensor_reduce(mu[:], x_ap, AX, OP.add)
    nc.scalar.mul(mu[:], mu[:], 1.0 / D)
    xc = lnp.tile([128, D], F32, name=f"xc_{name}", tag="xc")
    nc.vector.tensor_scalar_sub(xc[:], x_ap, mu[:])
    sq = lnp.tile([128, D], F32, name=f"sq_{name}", tag="sq")
    vs = lnp.tile([128, 1], F32, name=f"vs_{name}", tag="vs")
    nc.scalar.activation(sq[:], xc[:], ACT.Square, accum_out=vs[:])
    nc.scalar.activation(vs[:], vs[:], ACT.Copy, scale=1.0 / D, bias=1e-6)
    rv = lnp.tile([128, 1], F32, name=f"rv_{name}", tag="rv")
    nc.vector.reciprocal(rv[:], vs[:])
    rstd = lnp.tile([128, 1], F32, name=f"rstd_{name}", tag="rstd")
    nc.scalar.activation(rstd[:], rv[:], ACT.Sqrt)
    nc.vector.scalar_tensor_tensor(out_ap, xc[:], rstd[:], s_bc, OP.mult, OP.mult)
    nc.vector.tensor_add(out_ap, out_ap, b_bc)


def _tr(nc, pstr, name, src_ap, ident, outs):
    """PE-transpose a [128, <=128] block; copy the psum into each (ap, engine)."""
    p = pstr.tile([src_ap.shape[-1], 128], src_ap.dtype, name=f"tr_{name}",
                  tag="tr")
    nc.tensor.transpose(p[:], src_ap, ident)
    for ap, eng in outs:
        if eng == "v":
            nc.vector.tensor_copy(ap, p[:, :ap.shape[-1]])
        else:
            nc.scalar.copy(ap, p[:, :ap.shape[-1]])


def _feature(nc, fp, psf, fdram, nxT_r, routes, engs):
    """h[m] accumulators += w[:,n] * (nx @ f_n) for all 32 neurons.
    routes: list of (w_tiles_per_m, hacc_per_m). F is streamed in half-neuron
    chunks (4 k-tiles each) to halve SBUF residency."""
    HKT = KT // 2
    for n in range(N):
        pfs = [psf.tile([128, R], F32, name=f"pf{m}", tag=f"pf{m}")
               for m in range(MT)]
        for half in range(2):
            fc = fp.tile([128, HKT * R], fdram.dtype, name="fc", tag="fc")
            engs[(2 * n + half) % len(engs)].dma_start(
                fc[:], fdram[:, n, half * HKT * R:(half + 1) * HKT * R])
            for m in range(MT):
                for kk in range(HKT):
                    k = half * HKT + kk
                    nc.tensor.matmul(pfs[m][:],
                                     nxT_r[:, k, m * 128:(m + 1) * 128],
                                     fc[:, kk * R:(kk + 1) * R],
                                     start=(k == 0), stop=(k == KT - 1))
        for m in range(MT):
            for wt, hacc in routes:
                w_ap = wt[m][:, n:n + 1]
                if n == 0:
                    nc.vector.tensor_scalar(hacc[m][:], pfs[m][:], w_ap, None,
                                            OP.mult)
                else:
                    nc.vector.scalar_tensor_tensor(hacc[m][:], pfs[m][:], w_ap,
                                                   hacc[m][:], OP.mult, OP.add)


def _restore(nc, pools, rdram, routes, psy, engs):
    """Transposed-output restores sharing one streamed r matrix.
    routes: list of (hT_tile, wt_stage_dram, yT_out_ap).
    Packs len(routes) [128, TL] accumulators per PSUM bank; all matmuls use
    start=False on memset-zeroed banks."""
    nr = len(routes)
    assert nr in (1, 2)
    per_bank = 2
    n_banks = (KT * nr + per_bank - 1) // per_bank
    pys = [psy.tile([128, per_bank, TL], F32, name=f"ry{i}", tag=f"ry{i}")
           for i in range(n_banks)]
    for py in pys:
        nc.vector.memset(py[:], 0.0)

    def acc_ap(ri, dt):
        flat = dt * nr + ri
        return pys[flat // per_bank][:, flat % per_bank, :]

    rp, wbp, gtp = pools["rchunk"], pools["wb"], pools["gt"]
    for kt in range(NRT):
        n, rb = kt // 4, kt % 4
        wbs = []
        for ri, (hT, wtd, _) in enumerate(routes):
            if rb == 0:
                wb = wbp.tile([128, TL], F32, name=f"wb{ri}", tag=f"wb{ri}")
                nc.scalar.dma_start(wb[:],
                                    wtd[n:n + 1, :].broadcast_to([128, TL]))
                pools[f"_wb{ri}"] = wb
            wbs.append(pools[f"_wb{ri}"])
        if kt % 2 == 0:
            rc = rp.tile([128, 2, D], rdram.dtype, name="rc", tag="rc")
            engs[(kt // 2) % len(engs)].dma_start(rc[:], rdram[:, kt:kt + 2, :])
            pools["_rc"] = rc
        rc = pools["_rc"]
        for ri, (hT, _, _) in enumerate(routes):
            gt = gtp.tile([128, TL], rdram.dtype, name=f"gt{ri}", tag=f"gt{ri}")
            nc.vector.tensor_mul(gt[:], hT[:, rb, :], wbs[ri][:])
            for dt in range(KT):
                nc.tensor.matmul(acc_ap(ri, dt),
                                 rc[:, kt % 2, dt * 128:(dt + 1) * 128],
                                 gt[:], start=False, stop=(kt == NRT - 1))
    for ri, (_, _, yT_out) in enumerate(routes):
        for dt in range(KT):
            eng = nc.scalar if (dt + ri) % 2 == 0 else nc.vector
            if eng is nc.scalar:
                nc.scalar.copy(yT_out[:, dt, :], acc_ap(ri, dt))
            else:
                nc.vector.tensor_copy(yT_out[:, dt, :], acc_ap(ri, dt))


def build(dbg=False):
    nc = bacc.Bacc("TRN2", target_bir_lowering=False, debug=False,
                   num_devices=NCORES)

    x_d = nc.dram_tensor("x", [TL, D], F32, kind="ExternalInput")
    maskT_d = nc.dram_tensor("maskT", [S, TL], F32, kind="ExternalInput")
    wall_d = nc.dram_tensor("wall", [128, KT, 6 * DS], F32, kind="ExternalInput")
    wo_d = nc.dram_tensor("wo", [128, KT, D], F32, kind="ExternalInput")
    wfk_d = nc.dram_tensor("wfk", [128, KT, DS], F32, kind="ExternalInput")
    wrk_d = nc.dram_tensor("wrk", [128, KT, DS], F32, kind="ExternalInput")
    et_d = nc.dram_tensor("et", [DS, 6 * N], F32, kind="ExternalInput")
    fqk_d = nc.dram_tensor("fqk", [128, N, KT * R], BF16, kind="ExternalInput")
    fv_d = nc.dram_tensor("fv", [128, N, KT * R], BF16, kind="ExternalInput")
    fkn_d = nc.dram_tensor("fkn", [128, N, KT * R], BF16, kind="ExternalInput")
    rqk_d = nc.dram_tensor("rqk", [128, NRT, D], BF16, kind="ExternalInput")
    rv_d = nc.dram_tensor("rv", [128, NRT, D], BF16, kind="ExternalInput")
    rkn_d = nc.dram_tensor("rkn", [128, NRT, D], BF16, kind="ExternalInput")
    ln_d = nc.dram_tensor("lnrows", [4, D], F32, kind="ExternalInput")
    bias_d = nc.dram_tensor("biasrow", [1, 8 * DS], F32, kind="ExternalInput")
    y_d = nc.dram_tensor("y", [TL, D], F32, kind="ExternalOutput")

    dbg_t = {}

    def dbg_tensor(name, shape):
        dbg_t[name] = nc.dram_tensor("dbg_" + name, shape, F32,
                                     kind="ExternalOutput")
        return dbg_t[name]

    with tile.TileContext(nc) as tc:
        with (
            tc.tile_pool(name="perm", bufs=1) as perm,
            tc.tile_pool(name="dramp", bufs=1, space="DRAM") as dramp,
            tc.tile_pool(name="lnp", bufs=1) as lnp,
            tc.tile_pool(name="rtp", bufs=2) as rtp,
        ):
            # staging + collective buffers
            wt_stage = {k: dramp.tile([N, TL], F32, name=f"wt_{k}")
                        for k in ("rq", "rk", "rv", "rkn")}
            cc_in = dramp.tile([128, 2 * KT * TL], BF16, name="cc_in")
            cc_out = dramp.tile([4 * 128, 2 * KT * TL], BF16, name="cc_out")

            ident = perm.tile([128, 128], F32)
            make_identity(nc, ident[:])
            ident_b = perm.tile([128, 128], BF16)
            nc.vector.tensor_copy(ident_b[:], ident[:])
            ones_f = perm.tile([128, 1], F32)
            nc.gpsimd.memset(ones_f[:], 1.0)
            ones_b = perm.tile([128, 1], BF16)
            nc.vector.tensor_copy(ones_b[:], ones_f[:])
            bias_bc = perm.tile([128, 8 * DS], F32)
            nc.sync.dma_start(bias_bc[:], bias_d[0:1, :].broadcast_to([128, 8 * DS]))
            et_sb = perm.tile([DS, 6 * N], F32)
            nc.sync.dma_start(et_sb[:], et_d[:])
            # copy at partition base 64 for routings whose h-segment sits in
            # the upper half of a transposed tile (matmul requires equal bases)
            et_hi = perm.tile([128, 6 * N], F32)
            nc.sync.dma_start(et_hi[DS:2 * DS, :], et_d[:])
            x_sb = perm.tile([128, MT, D], F32)
            for m in range(MT):
                nc.sync.dma_start(x_sb[:, m, :], x_d[m * 128:(m + 1) * 128, :])
            yT_q = perm.tile([128, KT, TL], BF16)

            # ============ stage 1: LN1 + routing + features + restores ========
            with (
                tc.tile_pool(name="st1", bufs=1) as st1,
                tc.tile_pool(name="fchunk", bufs=2) as fp,
                tc.tile_pool(name="rchunk", bufs=2) as rp,
                tc.tile_pool(name="wbp", bufs=2) as wbp,
                tc.tile_pool(name="gtp", bufs=3) as gtp,
            ):
                nxT_r = st1.tile([128, KT, TL], BF16)
                h_q = [st1.tile([128, R], F32, name=f"h_q{m}") for m in range(MT)]
                h_k = [st1.tile([128, R], F32, name=f"h_k{m}") for m in range(MT)]
                h_v = [st1.tile([128, R], F32, name=f"h_v{m}") for m in range(MT)]
                hT = {k: st1.tile([128, 4, TL], F32, name=f"hT_{k}")
                      for k in ("q", "k", "v")}
                w_feat = {p: [st1.tile([128, N], F32, name=f"w{p}_{m}")
                              for m in range(MT)] for p in range(3)}
                yT_k = st1.tile([128, KT, TL], BF16)
                yT_v = st1.tile([128, KT, TL], BF16)
                v_tok = st1.tile([128, MT, D], BF16)

                with (
                    tc.tile_pool(name="st1a", bufs=1) as st1a,
                    tc.tile_pool(name="ps_tr", bufs=2, space="PSUM") as pstr,
                    tc.tile_pool(name="ps_mm", bufs=2, space="PSUM") as psmm,
                    tc.tile_pool(name="ps_feat", bufs=2, space="PSUM") as psf,
                    tc.tile_pool(name="wallp", bufs=2) as wallp,
                ):
                    nxT = st1a.tile([128, KT, TL], F32)
                    nx = st1a.tile([128, MT, D], F32)
                    ln1_bc = st1a.tile([128, 2, D], F32)
                    for i in range(2):
                        nc.sync.dma_start(ln1_bc[:, i, :],
                                          ln_d[i:i + 1, :].broadcast_to([128, D]))
                    for m in range(MT):
                        _layernorm(nc, lnp, f"ln1_{m}", x_sb[:, m, :],
                                   ln1_bc[:, 0, :], ln1_bc[:, 1, :], nx[:, m, :])
                    if dbg:
                        td = dbg_tensor("nx", [TL, D])
                        for m in range(MT):
                            nc.sync.dma_start(td[m * 128:(m + 1) * 128, :],
                                              nx[:, m, :])
                    for m in range(MT):
                        for k in range(KT):
                            _tr(nc, pstr, f"nx_{m}_{k}",
                                nx[:, m, k * 128:(k + 1) * 128], ident[:],
                                [(nxT[:, k, m * 128:(m + 1) * 128], "v"),
                                 (nxT_r[:, k, m * 128:(m + 1) * 128], "s")])

                    hall = st1a.tile([128, MT, 6 * DS], F32)
                    for m in range(MT):
                        ph = psmm.tile([128, 6 * DS], F32, name="ph", tag="mm")
                        for k in range(KT):
                            wt_k = wallp.tile([128, 6 * DS], F32, name="wal",
                                              tag="wal")
                            nc.sync.dma_start(wt_k[:], wall_d[:, k, :])
                            nc.tensor.matmul(ph[:],
                                             nxT[:, k, m * 128:(m + 1) * 128],
                                             wt_k[:],
                                             start=(k == 0), stop=(k == KT - 1))
                        nc.vector.tensor_add(hall[:, m, :], ph[:],
                                             bias_bc[:, :6 * DS])
                    hallT = st1a.tile([128, 3, TL], F32)
                    for m in range(MT):
                        for i in range(3):
                            _tr(nc, pstr, f"ha_{m}_{i}",
                                hall[:, m, i * 128:(i + 1) * 128], ident[:],
                                [(hallT[:, i, m * 128:(m + 1) * 128], "v")])
                    w_rest = {}
                    for p in range(6):
                        seg = ATTN_SEG[p]
                        tiles = w_feat[p] if p < 3 else \
                            [st1a.tile([128, N], F32, name=f"w{p}_{m}")
                             for m in range(MT)]
                        if p >= 3:
                            w_rest[p] = tiles
                        for m in range(MT):
                            base, ti = (p % 2) * DS, p // 2
                            e_src = et_sb if base == 0 else et_hi
                            e_ap = e_src[base:base + DS,
                                         seg * N:(seg + 1) * N]
                            _routing(nc, rtp, psmm, f"r{p}_{m}",
                                     hallT[base:base + DS, ti,
                                           m * 128:(m + 1) * 128],
                                     e_ap, tiles[m][:])
                    if dbg:
                        nm6 = ["w_fq", "w_fk", "w_fv", "w_rq", "w_rk", "w_rv"]
                        for p in range(6):
                            td = dbg_tensor(nm6[p], [TL, N])
                            tiles = w_feat[p] if p < 3 else w_rest[p]
                            for m in range(MT):
                                nc.sync.dma_start(td[m * 128:(m + 1) * 128, :],
                                                  tiles[m][:])
                    for p, key in [(3, "rq"), (4, "rk"), (5, "rv")]:
                        wtt = st1a.tile([N, TL], F32, name=f"wtt{p}")
                        for m in range(MT):
                            _tr(nc, pstr, f"wt_{p}_{m}", w_rest[p][m][:],
                                ident[:], [(wtt[:, m * 128:(m + 1) * 128], "v")])
                        nc.sync.dma_start(wt_stage[key][:], wtt[:])

                    # features (qk shared for Q and K; v)
                    _feature(nc, fp, psf, fqk_d, nxT_r,
                             [(w_feat[0], h_q), (w_feat[1], h_k)],
                             [nc.sync, nc.gpsimd])
                    _feature(nc, fp, psf, fv_d, nxT_r,
                             [(w_feat[2], h_v)], [nc.sync, nc.gpsimd])
                    if dbg:
                        for nm, hh in [("h_q", h_q), ("h_k", h_k), ("h_v", h_v)]:
                            td = dbg_tensor(nm, [TL, R])
                            for m in range(MT):
                                nc.sync.dma_start(td[m * 128:(m + 1) * 128, :],
                                                  hh[m][:])
                    for nm, hh in [("q", h_q), ("k", h_k), ("v", h_v)]:
                        for m in range(MT):
                            for rb in range(4):
                                _tr(nc, pstr, f"h{nm}_{m}_{rb}",
                                    hh[m][:, rb * 128:(rb + 1) * 128], ident[:],
                                    [(hT[nm][:, rb, m * 128:(m + 1) * 128], "v")])

                # restores: Q+K fused (stream r_qk once), then V
                pools = {"rchunk": rp, "wb": wbp, "gt": gtp}
                with tc.tile_pool(name="ps_y", bufs=1, space="PSUM") as psy:
                    _restore(nc, pools, rqk_d,
                             [(hT["q"][:], wt_stage["rq"], yT_q[:]),
                              (hT["k"][:], wt_stage["rk"], yT_k[:])],
                             psy, [nc.sync, nc.gpsimd])
                with tc.tile_pool(name="ps_y2", bufs=1, space="PSUM") as psy:
                    _restore(nc, pools, rv_d,
                             [(hT["v"][:], wt_stage["rv"], yT_v[:])],
                             psy, [nc.sync, nc.gpsimd])
                if dbg:
                    for nm, yy in [("yT_q", yT_q), ("yT_k", yT_k),
                                   ("yT_v", yT_v)]:
                        td = dbg_tensor(nm, [D, TL])
                        for dt in range(KT):
                            nc.gpsimd.dma_start(td[dt * 128:(dt + 1) * 128, :],
                                                yy[:, dt, :])

                with tc.tile_pool(name="ps_tr2", bufs=2, space="PSUM") as pstr2:
                    for dt in range(KT):
                        for m in range(MT):
                            _tr(nc, pstr2, f"v_{dt}_{m}",
                                yT_v[:, dt, m * 128:(m + 1) * 128], ident_b[:],
                                [(v_tok[:, m, dt * 128:(dt + 1) * 128], "s")])
                nc.sync.dma_start(cc_in[:, :KT * TL],
                                  yT_k[:].rearrange("p k t -> p (k t)"))
                for m in range(MT):
                    nc.sync.dma_start(
                        cc_in[:, KT * TL + m * D:KT * TL + (m + 1) * D],
                        v_tok[:, m, :])
            nc.gpsimd.collective_compute(
                "AllGather", OP.bypass,
                ins=[cc_in[:]],
                outs=[cc_out[:]],
                replica_groups=[[0, 1, 2, 3], [4, 5, 6, 7]],
            )

            # ============ stage 2: attention + W_o ============
            late_cm = tc.tile_pool(name="late", bufs=1)
            late = late_cm.__enter__()
            x2 = late.tile([128, MT, D], F32)
            ot_sb = late.tile([128, KT, TL], F32)
            with (
                tc.tile_pool(name="st2", bufs=1) as st2,
                tc.tile_pool(name="attp", bufs=3) as att,
                tc.tile_pool(name="ps_att", bufs=2, space="PSUM") as psa,
                tc.tile_pool(name="ps_ot", bufs=4, space="PSUM") as psot,
            ):
                maskT_sb = st2.tile([128, SEQ_BLOCKS, TL], F32)
                for kb in range(SEQ_BLOCKS):
                    nc.sync.dma_start(maskT_sb[:, kb, :],
                                      maskT_d[kb * 128:(kb + 1) * 128, :])
                kt_all = st2.tile([128, 4, KT * TL], BF16)
                v_all = st2.tile([128, 4, MT * D], BF16)
                for ch in range(4):
                    nc.sync.dma_start(kt_all[:, ch, :],
                                      cc_out[ch * 128:(ch + 1) * 128, :KT * TL])
                    nc.sync.dma_start(v_all[:, ch, :],
                                      cc_out[ch * 128:(ch + 1) * 128, KT * TL:])
                for hp in range(KT):
                    for hh in range(2):
                        pot = psot.tile([DH, TL], F32, name="pot", tag="pot")
                        nc.vector.memset(pot[:], 0.0)
                        h_idx = hp * 2 + hh
                        qt_ap = yT_q[hh * DH:(hh + 1) * DH, hp, :]
                        pss = psa.tile([1, TL], F32, name="pss", tag="pss")
                        nc.vector.memset(pss[:], 0.0)
                        for kb in range(SEQ_BLOCKS):
                            ch, m2 = kb // 2, kb % 2
                            ktap = kt_all[hh * DH:(hh + 1) * DH, ch,
                                          hp * TL + m2 * 128:
                                          hp * TL + (m2 + 1) * 128]
                            vap = v_all[:, ch,
                                        m2 * D + h_idx * DH:
                                        m2 * D + (h_idx + 1) * DH]
                            pscore = psa.tile([128, TL], F32, name="pscore",
                                              tag="pscore")
                            nc.tensor.matmul(pscore[:], ktap, qt_ap,
                                             start=True, stop=True)
                            msc = att.tile([128, TL], F32, name="msc", tag="msc")
                            nc.vector.tensor_add(msc[:], pscore[:],
                                                 maskT_sb[:, kb, :])
                            expt = att.tile([128, TL], BF16, name="expt",
                                            tag="expt")
                            nc.scalar.activation(expt[:], msc[:], ACT.Exp,
                                                 scale=0.125)
                            nc.tensor.matmul(pss[:], ones_b[:], expt[:],
                                             start=False,
                                             stop=(kb == SEQ_BLOCKS - 1))
                            nc.tensor.matmul(pot[:], vap, expt[:], start=False,
                                             stop=(kb == SEQ_BLOCKS - 1))
                        rsr = att.tile([1, TL], F32, name="rsr", tag="rsr")
                        nc.vector.reciprocal(rsr[:], pss[:])
                        rbc = att.tile([DH, TL], F32, name="rbc", tag="rbc")
                        nc.gpsimd.partition_broadcast(rbc[:], rsr[:], channels=DH)
                        otn = att.tile([DH, TL], F32, name="otn", tag="otn")
                        nc.vector.tensor_mul(otn[:], pot[:], rbc[:])
                        # SBUF->SBUF DMA can shift partitions (DVE cannot)
                        nc.sync.dma_start(ot_sb[hh * DH:(hh + 1) * DH, hp, :],
                                          otn[:])
                if dbg:
                    td = dbg_tensor("oT", [D, TL])
                    for dt in range(KT):
                        nc.sync.dma_start(td[dt * 128:(dt + 1) * 128, :],
                                          ot_sb[:, dt, :])

            with (
                tc.tile_pool(name="wop", bufs=3) as wop,
                tc.tile_pool(name="ps_mm2", bufs=2, space="PSUM") as psmm2,
            ):
                for blk in range(2):
                    wo_t = []
                    for k in range(KT):
                        wt_k = wop.tile([128, 512], F32, name=f"wo{k}", tag="wo")
                        nc.sync.dma_start(wt_k[:],
                                          wo_d[:, k, blk * 512:(blk + 1) * 512])
                        wo_t.append(wt_k)
                    for m in range(MT):
                        px = psmm2.tile([128, 512], F32, name="px", tag="mm")
                        for k in range(KT):
                            nc.tensor.matmul(px[:],
                                             ot_sb[:, k, m * 128:(m + 1) * 128],
                                             wo_t[k][:],
                                             start=(k == 0), stop=(k == KT - 1))
                        nc.vector.tensor_add(
                            x2[:, m, blk * 512:(blk + 1) * 512], px[:],
                            x_sb[:, m, blk * 512:(blk + 1) * 512])
            if dbg:
                td = dbg_tensor("x2", [TL, D])
                for m in range(MT):
                    nc.sync.dma_start(td[m * 128:(m + 1) * 128, :], x2[:, m, :])

            # ============ stage 3: knowledge circuit ============
            with (
                tc.tile_pool(name="st3", bufs=1) as st3,
                tc.tile_pool(name="fchunk2", bufs=2) as fp2,
                tc.tile_pool(name="rchunk2", bufs=2) as rp2,
                tc.tile_pool(name="wbp2", bufs=2) as wbp2,
                tc.tile_pool(name="gtp2", bufs=3) as gtp2,
            ):
                nx2T_r = st3.tile([128, KT, TL], BF16)
                h_kn = [st3.tile([128, R], F32, name=f"h_kn{m}")
                        for m in range(MT)]
                hT_kn = st3.tile([128, 4, TL], F32)
                w_kn = {}
                yT_kn = st3.tile([128, KT, TL], F32)
                with (
                    tc.tile_pool(name="st3a", bufs=1) as st3a,
                    tc.tile_pool(name="ps_tr3", bufs=2, space="PSUM") as pstr3,
                    tc.tile_pool(name="ps_mm3", bufs=2, space="PSUM") as psmm3,
                    tc.tile_pool(name="ps_feat3", bufs=2, space="PSUM") as psf3,
                ):
                    nx2 = st3a.tile([128, MT, D], F32)
                    ln2_bc = st3a.tile([128, 2, D], F32)
                    for i in range(2):
                        nc.sync.dma_start(
                            ln2_bc[:, i, :],
                            ln_d[i + 2:i + 3, :].broadcast_to([128, D]))
                    for m in range(MT):
                        _layernorm(nc, lnp, f"ln2_{m}", x2[:, m, :],
                                   ln2_bc[:, 0, :], ln2_bc[:, 1, :], nx2[:, m, :])
                    nx2T = st3a.tile([128, KT, TL], F32)
                    for m in range(MT):
                        for k in range(KT):
                            _tr(nc, pstr3, f"nx2_{m}_{k}",
                                nx2[:, m, k * 128:(k + 1) * 128], ident[:],
                                [(nx2T[:, k, m * 128:(m + 1) * 128], "v"),
                                 (nx2T_r[:, k, m * 128:(m + 1) * 128], "s")])
                    wk_sb = st3a.tile([128, KT, 2 * DS], F32)
                    nc.sync.dma_start(wk_sb[:, :, :DS], wfk_d[:])
                    nc.sync.dma_start(wk_sb[:, :, DS:], wrk_d[:])
                    hkT = st3a.tile([DS, 2, TL], F32)
                    for m in range(MT):
                        for j in range(2):
                            pk = psmm3.tile([128, DS], F32, name="pk", tag="mm")
                            for k in range(KT):
                                nc.tensor.matmul(
                                    pk[:], nx2T[:, k, m * 128:(m + 1) * 128],
                                    wk_sb[:, k, j * DS:(j + 1) * DS],
                                    start=(k == 0), stop=(k == KT - 1))
                            hk = rtp.tile([128, DS], F32, name=f"hk{m}{j}",
                                          tag="hk")
                            nc.vector.tensor_add(
                                hk[:], pk[:],
                                bias_bc[:, (6 + j) * DS:(7 + j) * DS])
                            _tr(nc, pstr3, f"hk_{m}_{j}", hk[:], ident[:],
                                [(hkT[:, j, m * 128:(m + 1) * 128], "v")])
                    for j, nm in [(0, "fkn"), (1, "rkn")]:
                        w_kn[nm] = []
                        for m in range(MT):
                            wt = st3.tile([128, N], F32, name=f"wkn{j}_{m}")
                            _routing(nc, rtp, psmm3, f"rk{j}_{m}",
                                     hkT[:, j, m * 128:(m + 1) * 128],
                                     et_sb[:, (4 + j) * N:(5 + j) * N], wt[:])
                            w_kn[nm].append(wt)
                    if dbg:
                        for nm, key in [("w_fknow", "fkn"), ("w_rknow", "rkn")]:
                            td = dbg_tensor(nm, [TL, N])
                            for m in range(MT):
                                nc.sync.dma_start(td[m * 128:(m + 1) * 128, :],
                                                  w_kn[key][m][:])
                    wtt = st3a.tile([N, TL], F32, name="wtt_kn")
                    for m in range(MT):
                        _tr(nc, pstr3, f"wt_kn_{m}", w_kn["rkn"][m][:],
                            ident[:], [(wtt[:, m * 128:(m + 1) * 128], "v")])
                    nc.sync.dma_start(wt_stage["rkn"][:], wtt[:])

                    _feature(nc, fp2, psf3, fkn_d, nx2T_r,
                             [(w_kn["fkn"], h_kn)], [nc.sync, nc.gpsimd])
                    if dbg:
                        td = dbg_tensor("h_know", [TL, R])
                        for m in range(MT):
                            nc.sync.dma_start(td[m * 128:(m + 1) * 128, :],
                                              h_kn[m][:])
                    for m in range(MT):
                        for rb in range(4):
                            _tr(nc, pstr3, f"hkn_{m}_{rb}",
                                h_kn[m][:, rb * 128:(rb + 1) * 128], ident[:],
                                [(hT_kn[:, rb, m * 128:(m + 1) * 128], "v")])

                pools3 = {"rchunk": rp2, "wb": wbp2, "gt": gtp2}
                with tc.tile_pool(name="ps_y3", bufs=1, space="PSUM") as psy3:
                    _restore(nc, pools3, rkn_d,
                             [(hT_kn[:], wt_stage["rkn"], yT_kn[:])],
                             psy3, [nc.sync, nc.gpsimd])

                out_sb = st3.tile([128, MT, D], F32)
                with tc.tile_pool(name="ps_fin", bufs=2, space="PSUM") as psfin:
                    for dt in range(KT):
                        for m in range(MT):
                            p = psfin.tile([128, 128], F32,
                                           name=f"fin_{dt}_{m}", tag="fin")
                            nc.tensor.transpose(
                                p[:], yT_kn[:, dt, m * 128:(m + 1) * 128],
                                ident[:])
                            nc.vector.tensor_add(
                                out_sb[:, m, dt * 128:(dt + 1) * 128], p[:],
                                x2[:, m, dt * 128:(dt + 1) * 128])
                for m in range(MT):
                    nc.sync.dma_start(y_d[m * 128:(m + 1) * 128, :],
                                      out_sb[:, m, :])
            late_cm.__exit__(None, None, None)

    nc.compile()
    return nc, dbg_t


def prep_inputs(inputs):
    f32 = np.float32
    bf16 = mybir.dt.np(BF16)
    x = np.ascontiguousarray(np.asarray(inputs["x"], f32).reshape(T, D))
    ne = np.asarray(inputs["neuron_emb"], f32)
    emb = ne / (np.linalg.norm(ne, axis=-1, keepdims=True) + 1e-8)

    def f_layout(f):
        f = np.asarray(f, f32)
        return np.ascontiguousarray(
            f.reshape(N, KT, 128, R).transpose(2, 0, 1, 3)
            .reshape(128, N, KT * R).astype(bf16))

    def r_layout(r):
        r = np.asarray(r, f32).reshape(N * R, D)
        return np.ascontiguousarray(
            r.reshape(NRT, 128, D).transpose(1, 0, 2).astype(bf16))

    def w_layout(w):
        w = np.asarray(w, f32)
        return np.ascontiguousarray(
            w.reshape(KT, 128, w.shape[-1]).transpose(1, 0, 2))

    shared = {
        "wall": w_layout(inputs["W_all"]),
        "wo": w_layout(inputs["W_o"]),
        "wfk": w_layout(inputs["W_fk"]),
        "wrk": w_layout(inputs["W_rk"]),
        "et": np.ascontiguousarray(emb.T),
        "fqk": f_layout(inputs["f_qk"]),
        "fv": f_layout(inputs["f_v"]),
        "fkn": f_layout(inputs["f_know"]),
        "rqk": r_layout(inputs["r_qk"]),
        "rv": r_layout(inputs["r_v"]),
        "rkn": r_layout(inputs["r_know"]),
        "lnrows": np.ascontiguousarray(
            np.stack([np.asarray(inputs[k], f32)
                      for k in ("ln1_s", "ln1_b", "ln2_s", "ln2_b")])),
        "biasrow": np.ascontiguousarray(
            np.concatenate([np.asarray(inputs["b_all"], f32),
                            np.asarray(inputs["b_fk"], f32),
                            np.asarray(inputs["b_rk"], f32)])[None, :]),
    }
    per_core = []
    k_idx = np.arange(S)[:, None]
    for c in range(NCORES):
        ci = c % (S // TL)
        q_idx = ci * TL + np.arange(TL)[None, :]
        maskT = np.where(k_idx <= q_idx, 0.0, NEG).astype(f32)
        per_core.append({
            "x": np.ascontiguousarray(x[c * TL:(c + 1) * TL]),
            "maskT": np.ascontiguousarray(maskT),
            **shared,
        })
    return per_core


def kernel(**inputs):
    global _PROG
    if _PROG is None:
        _PROG = build(dbg=False)
    nc, _ = _PROG
    per_core = prep_inputs(inputs)
    res = run_bass_kernel_spmd(nc, per_core, core_ids=list(range(NCORES)))
    y = np.concatenate([res.results[c]["y"] for c in range(NCORES)], axis=0)
    return y.reshape(B, S, D).astype(np.float32)



# revision 48
# speedup vs baseline: 1.8085x; 1.2705x over previous
"""Trainium2 Bass kernel for the moe_routing problem (nn_DAWN_69904887709893).

Token-parallel across 8 NeuronCores (256 tokens/core), neuron pools replicated.
Heavy einsums (feature/restore/attention) run in fp32r on the PE; every matmul
feeding router logits (W_all, logit projections, W_fk/W_rk, W_o) runs in plain
fp32 so top-k selections match the fp32 reference. K^T and token-major V are
exchanged with a 4-rank AllGather per sequence group; causality comes from a
host-provided additive mask so the SPMD program is identical on every core.

PSUM accumulators that pack two regions per bank are pre-zeroed with memset and
use start=False matmuls throughout: correct regardless of has_written state and
immune to instruction reordering (a start=True clears the whole bank's bits).
"""
import sys

sys.path.insert(0, "/opt/trn_rl_repo")
import numpy as np
import concourse.bass as bass
import concourse.bacc as bacc
import concourse.mybir as mybir
import concourse.tile as tile
from concourse.bass_utils import run_bass_kernel_spmd
from concourse.masks import make_identity

F32 = mybir.dt.float32
F32R = mybir.dt.float32r
BF16 = mybir.dt.bfloat16
AX = mybir.AxisListType.X
OP = mybir.AluOpType
ACT = mybir.ActivationFunctionType

NCORES = 8
B, S, D, R, N, DS, TOPK, H = 2, 1024, 1024, 512, 32, 64, 4, 16
T = B * S
TL = T // NCORES          # tokens per core (256)
MT = TL // 128            # token tiles per core (2)
KT = D // 128             # contraction tiles over D (8)
DH = D // H               # head dim (64)
NRT = (N * R) // 128      # contraction tiles over N*R (128)
SEQ_BLOCKS = S // 128     # k blocks per sequence (8)
NEG = -1.0e30

# emb segment used by each of the 6 attention routings (fq, fk, fv, rq, rk, rv)
ATTN_SEG = [0, 0, 1, 2, 2, 3]

_PROG = None


def _routing(nc, rt, psmm, name, lhsT_ap, e_ap, wout):
    """softmax over 32 logits + top-4 sparsify + renormalize -> wout [128,32] f32.
    lhsT_ap: [64, 128] fp32 (h-segment transposed), e_ap: [64, 32] fp32."""
    lg = psmm.tile([128, N], F32, name=f"lg_{name}", tag="mm")
    nc.tensor.matmul(lg[:], lhsT_ap, e_ap, start=True, stop=True)
    mx = rt.tile([128, 1], F32, name=f"mx_{name}", tag="mx")
    nc.vector.tensor_reduce(mx[:], lg[:], AX, OP.max)
    nmx = rt.tile([128, 1], F32, name=f"nmx_{name}", tag="nmx")
    nc.scalar.mul(nmx[:], mx[:], -1.0)
    ex = rt.tile([128, N], F32, name=f"ex_{name}", tag="ex")
    ssum = rt.tile([128, 1], F32, name=f"ssum_{name}", tag="ssum")
    nc.scalar.activation(ex[:], lg[:], ACT.Exp, bias=nmx[:], scale=1.0,
                         accum_out=ssum[:])
    rs = rt.tile([128, 1], F32, name=f"rs_{name}", tag="rs")
    nc.vector.reciprocal(rs[:], ssum[:])
    sm = rt.tile([128, N], F32, name=f"sm_{name}", tag="sm")
    nc.vector.tensor_scalar_mul(sm[:], ex[:], rs[:])
    top8 = rt.tile([128, 8], F32, name=f"top8_{name}", tag="top8")
    nc.vector.max(top8[:], sm[:])
    ge = rt.tile([128, N], F32, name=f"ge_{name}", tag="ge")
    nc.vector.tensor_scalar(ge[:], sm[:], top8[:, 3:4], None, OP.is_ge)
    sp = rt.tile([128, N], F32, name=f"sp_{name}", tag="sp")
    nc.vector.tensor_mul(sp[:], sm[:], ge[:])
    s2 = rt.tile([128, 1], F32, name=f"s2_{name}", tag="s2")
    nc.vector.tensor_reduce(s2[:], sp[:], AX, OP.add)
    s2e = rt.tile([128, 1], F32, name=f"s2e_{name}", tag="s2e")
    nc.vector.tensor_scalar_add(s2e[:], s2[:], 1e-8)
    rs2 = rt.tile([128, 1], F32, name=f"rs2_{name}", tag="rs2")
    nc.vector.reciprocal(rs2[:], s2e[:])
    nc.vector.tensor_scalar_mul(wout, sp[:], rs2[:])


def _layernorm(nc, lnp, name, x_ap, s_bc, b_bc, out_ap):
    """LN over the free dim (D). x_ap/out_ap [128, D] f32; s_bc/b_bc [128, D]."""
    mu = lnp.tile([128, 1], F32, name=f"mu_{name}", tag="mu")
    nc.vector.tensor_reduce(mu[:], x_ap, AX, OP.add)
    nc.scalar.mul(mu[:], mu[:], 1.0 / D)
    xc = lnp.tile([128, D], F32, name=f"xc_{name}", tag="xc")
    nc.vector.tensor_scalar_sub(xc[:], x_ap, mu[:])
    sq = lnp.tile([128, D], F32, name=f"sq_{name}", tag="sq")
    vs = lnp.tile([128, 1], F32, name=f"vs_{name}", tag="vs")
    nc.scalar.activation(sq[:], xc[:], ACT.Square, accum_out=vs[:])
    nc.scalar.activation(vs[:], vs[:], ACT.Copy, scale=1.0 / D, bias=1e-6)
    rv = lnp.tile([128, 1], F32, name=f"rv_{name}", tag="rv")
    nc.vector.reciprocal(rv[:], vs[:])
    rstd = lnp.tile([128, 1], F32, name=f"rstd_{name}", tag="rstd")
    nc.scalar.activation(rstd[:], rv[:], ACT.Sqrt)
    nc.vector.scalar_tensor_tensor(out_ap, xc[:], rstd[:], s_bc, OP.mult, OP.mult)
    nc.vector.tensor_add(out_ap, out_ap, b_bc)


def _tr(nc, pstr, name, src_ap, ident, outs):
    """PE-transpose a [128, <=128] block; copy the psum into each (ap, engine)."""
    p = pstr.tile([src_ap.shape[-1], 128], src_ap.dtype, name=f"tr_{name}",
                  tag="tr")
    nc.tensor.transpose(p[:], src_ap, ident)
    for ap, eng in outs:
        if eng == "v":
            nc.vector.tensor_copy(ap, p[:, :ap.shape[-1]])
        else:
            nc.scalar.copy(ap, p[:, :ap.shape[-1]])


def _feature(nc, fp, psf, fdram, nxT_r, routes, engs):
    """h[m] accumulators += w[:,n] * (nx @ f_n) for all 32 neurons.
    routes: list of (w_tiles_per_m, hacc_per_m). F is streamed in half-neuron
    chunks (4 k-tiles each) to halve SBUF residency."""
    HKT = KT // 2
    for n in range(N):
        pfs = [psf.tile([128, R], F32, name=f"pf{m}", tag=f"pf{m}")
               for m in range(MT)]
        for half in range(2):
            fc = fp.tile([128, HKT * R], fdram.dtype, name="fc", tag="fc")
            engs[(2 * n + half) % len(engs)].dma_start(
                fc[:], fdram[:, n, half * HKT * R:(half + 1) * HKT * R])
            for m in range(MT):
                for kk in range(HKT):
                    k = half * HKT + kk
                    nc.tensor.matmul(pfs[m][:],
                                     nxT_r[:, k, m * 128:(m + 1) * 128],
                                     fc[:, kk * R:(kk + 1) * R],
                                     start=(k == 0), stop=(k == KT - 1))
        for m in range(MT):
            for wt, hacc in routes:
                w_ap = wt[m][:, n:n + 1]
                if n == 0:
                    nc.vector.tensor_scalar(hacc[m][:], pfs[m][:], w_ap, None,
                                            OP.mult)
                else:
                    nc.vector.scalar_tensor_tensor(hacc[m][:], pfs[m][:], w_ap,
                                                   hacc[m][:], OP.mult, OP.add)


def _restore(nc, pools, rdram, routes, psy, engs):
    """Transposed-output restores sharing one streamed r matrix.
    routes: list of (hT_tile, w_sb [N, TL] sbuf tile, yT_out_ap).
    Packs len(routes) [128, TL] accumulators per PSUM bank; all matmuls use
    start=False on memset-zeroed banks."""
    nr = len(routes)
    assert nr in (1, 2)
    per_bank = 2
    n_banks = (KT * nr + per_bank - 1) // per_bank
    pys = [psy.tile([128, per_bank, TL], F32, name=f"ry{i}", tag=f"ry{i}")
           for i in range(n_banks)]
    for py in pys:
        nc.vector.memset(py[:], 0.0)

    def acc_ap(ri, dt):
        flat = dt * nr + ri
        return pys[flat // per_bank][:, flat % per_bank, :]

    rp, wbp, gtp = pools["rchunk"], pools["wb"], pools["gt"]
    for kt in range(NRT):
        n, rb = kt // 4, kt % 4
        wbs = []
        for ri, (hT, wtd, _) in enumerate(routes):
            if rb == 0:
                wb = wbp.tile([128, TL], F32, name=f"wb{ri}", tag=f"wb{ri}")
                nc.gpsimd.dma_start(wb[:],
                                    wtd[n:n + 1, :].broadcast_to([128, TL]))
                pools[f"_wb{ri}"] = wb
            wbs.append(pools[f"_wb{ri}"])
        if kt % 2 == 0:
            rc = rp.tile([128, 2, D], rdram.dtype, name="rc", tag="rc")
            engs[(kt // 2) % len(engs)].dma_start(rc[:], rdram[:, kt:kt + 2, :])
            pools["_rc"] = rc
        rc = pools["_rc"]
        for ri, (hT, _, _) in enumerate(routes):
            gt = gtp.tile([128, TL], rdram.dtype, name=f"gt{ri}", tag=f"gt{ri}")
            nc.vector.tensor_mul(gt[:], hT[:, rb, :], wbs[ri][:])
            for dt in range(KT):
                nc.tensor.matmul(acc_ap(ri, dt),
                                 rc[:, kt % 2, dt * 128:(dt + 1) * 128],
                                 gt[:], start=False, stop=(kt == NRT - 1))
    for ri, (_, _, yT_out) in enumerate(routes):
        for dt in range(KT):
            eng = nc.scalar if (dt + ri) % 2 == 0 else nc.vector
            if eng is nc.scalar:
                nc.scalar.copy(yT_out[:, dt, :], acc_ap(ri, dt))
            else:
                nc.vector.tensor_copy(yT_out[:, dt, :], acc_ap(ri, dt))


def build(dbg=False):
    nc = bacc.Bacc("TRN2", target_bir_lowering=False, debug=False,
                   num_devices=NCORES)

    x_d = nc.dram_tensor("x", [TL, D], F32, kind="ExternalInput")
    maskT_d = nc.dram_tensor("maskT", [S, TL], F32, kind="ExternalInput")
    wall_d = nc.dram_tensor("wall", [128, KT, 6 * DS], F32, kind="ExternalInput")
    wo_d = nc.dram_tensor("wo", [128, KT, D], F32R, kind="ExternalInput")
    wfk_d = nc.dram_tensor("wfk", [128, KT, DS], F32, kind="ExternalInput")
    wrk_d = nc.dram_tensor("wrk", [128, KT, DS], F32, kind="ExternalInput")
    et_d = nc.dram_tensor("et", [DS, 6 * N], F32, kind="ExternalInput")
    fqk_d = nc.dram_tensor("fqk", [128, N, KT * R], BF16, kind="ExternalInput")
    fv_d = nc.dram_tensor("fv", [128, N, KT * R], BF16, kind="ExternalInput")
    fkn_d = nc.dram_tensor("fkn", [128, N, KT * R], BF16, kind="ExternalInput")
    rqk_d = nc.dram_tensor("rqk", [128, NRT, D], BF16, kind="ExternalInput")
    rv_d = nc.dram_tensor("rv", [128, NRT, D], BF16, kind="ExternalInput")
    rkn_d = nc.dram_tensor("rkn", [128, NRT, D], BF16, kind="ExternalInput")
    ln_d = nc.dram_tensor("lnrows", [4, D], F32, kind="ExternalInput")
    bias_d = nc.dram_tensor("biasrow", [1, 8 * DS], F32, kind="ExternalInput")
    y_d = nc.dram_tensor("y", [TL, D], F32, kind="ExternalOutput")

    dbg_t = {}

    def dbg_tensor(name, shape):
        dbg_t[name] = nc.dram_tensor("dbg_" + name, shape, F32,
                                     kind="ExternalOutput")
        return dbg_t[name]

    with tile.TileContext(nc) as tc:
        with (
            tc.tile_pool(name="perm", bufs=1) as perm,
            tc.tile_pool(name="dramp", bufs=1, space="DRAM") as dramp,
            tc.tile_pool(name="lnp", bufs=1) as lnp,
            tc.tile_pool(name="rtp", bufs=2) as rtp,
        ):
            # collective bounce buffers (K^T gathered first, then token-major V)
            cck_in = dramp.tile([128, KT * TL], BF16, name="cck_in")
            cck_out = dramp.tile([4 * 128, KT * TL], BF16, name="cck_out")
            ccv_in = dramp.tile([128, MT * D], BF16, name="ccv_in")
            ccv_out = dramp.tile([4 * 128, MT * D], BF16, name="ccv_out")
            # restore-route w rows, bounced through DRAM into one partition
            wt_dram = {k: dramp.tile([1, N, TL], F32, name=f"wtd_{k}")
                       for k in ("rq", "rk", "rv", "rkn")}

            ident = perm.tile([128, 128], F32)
            make_identity(nc, ident[:])
            ident_b = perm.tile([128, 128], BF16)
            nc.vector.tensor_copy(ident_b[:], ident[:])
            ones_f = perm.tile([128, 1], F32)
            nc.gpsimd.memset(ones_f[:], 1.0)
            ones_b = perm.tile([128, 1], BF16)
            nc.vector.tensor_copy(ones_b[:], ones_f[:])
            bias_bc = perm.tile([128, 8 * DS], F32)
            nc.sync.dma_start(bias_bc[:], bias_d[0:1, :].broadcast_to([128, 8 * DS]))
            et_sb = perm.tile([DS, 6 * N], F32)
            nc.sync.dma_start(et_sb[:], et_d[:])
            # copy at partition base 64 for routings whose h-segment sits in
            # the upper half of a transposed tile (matmul requires equal bases)
            et_hi = perm.tile([128, 6 * N], F32)
            nc.sync.dma_start(et_hi[DS:2 * DS, :], et_d[:])
            x_sb = perm.tile([128, MT, D], F32)
            for m in range(MT):
                nc.sync.dma_start(x_sb[:, m, :], x_d[m * 128:(m + 1) * 128, :])
            yT_q = perm.tile([128, KT, TL], BF16)

            # ============ stage 1: LN1 + routing + features + restores ========
            with (
                tc.tile_pool(name="st1", bufs=1) as st1,
                tc.tile_pool(name="fchunk", bufs=3) as fp,
                tc.tile_pool(name="rchunk", bufs=3) as rp,
                tc.tile_pool(name="wbp", bufs=2) as wbp,
                tc.tile_pool(name="gtp", bufs=3) as gtp,
            ):
                nxT_r = st1.tile([128, KT, TL], BF16)
                h_q = [st1.tile([128, R], F32, name=f"h_q{m}") for m in range(MT)]
                h_k = [st1.tile([128, R], F32, name=f"h_k{m}") for m in range(MT)]
                h_v = [st1.tile([128, R], F32, name=f"h_v{m}") for m in range(MT)]
                hT = {k: st1.tile([128, 4, TL], F32, name=f"hT_{k}")
                      for k in ("q", "k", "v")}
                w_feat = {p: [st1.tile([128, N], F32, name=f"w{p}_{m}")
                              for m in range(MT)] for p in range(3)}
                wtt_sb = {k: st1.tile([N, TL], F32, name=f"wtt_{k}")
                          for k in ("rq", "rk", "rv")}
                yT_k = st1.tile([128, KT, TL], BF16)
                yT_v = st1.tile([128, KT, TL], BF16)
                v_tok = st1.tile([128, MT, D], BF16)

                with (
                    tc.tile_pool(name="st1a", bufs=1) as st1a,
                    tc.tile_pool(name="ps_tr", bufs=2, space="PSUM") as pstr,
                    tc.tile_pool(name="ps_mm", bufs=2, space="PSUM") as psmm,
                    tc.tile_pool(name="ps_feat", bufs=2, space="PSUM") as psf,
                    tc.tile_pool(name="wallp", bufs=2) as wallp,
                ):
                    nxT = st1a.tile([128, KT, TL], F32)
                    nx = st1a.tile([128, MT, D], F32)
                    ln1_bc = st1a.tile([128, 2, D], F32)
                    for i in range(2):
                        nc.sync.dma_start(ln1_bc[:, i, :],
                                          ln_d[i:i + 1, :].broadcast_to([128, D]))
                    for m in range(MT):
                        _layernorm(nc, lnp, f"ln1_{m}", x_sb[:, m, :],
                                   ln1_bc[:, 0, :], ln1_bc[:, 1, :], nx[:, m, :])
                    if dbg:
                        td = dbg_tensor("nx", [TL, D])
                        for m in range(MT):
                            nc.sync.dma_start(td[m * 128:(m + 1) * 128, :],
                                              nx[:, m, :])
                    for m in range(MT):
                        for k in range(KT):
                            _tr(nc, pstr, f"nx_{m}_{k}",
                                nx[:, m, k * 128:(k + 1) * 128], ident[:],
                                [(nxT[:, k, m * 128:(m + 1) * 128], "v"),
                                 (nxT_r[:, k, m * 128:(m + 1) * 128], "s")])

                    hall = st1a.tile([128, MT, 6 * DS], F32)
                    phs = [psmm.tile([128, 6 * DS], F32, name=f"ph{m}",
                                     tag="mm") for m in range(MT)]
                    for k in range(KT):
                        wt_k = wallp.tile([128, 6 * DS], F32, name="wal",
                                          tag="wal")
                        [nc.sync, nc.scalar][k % 2].dma_start(
                            wt_k[:], wall_d[:, k, :])
                        for m in range(MT):
                            nc.tensor.matmul(phs[m][:],
                                             nxT[:, k, m * 128:(m + 1) * 128],
                                             wt_k[:],
                                             start=(k == 0), stop=(k == KT - 1))
                    for m in range(MT):
                        nc.vector.tensor_add(hall[:, m, :], phs[m][:],
                                             bias_bc[:, :6 * DS])
                    hallT = st1a.tile([128, 3, TL], F32)
                    for m in range(MT):
                        for i in range(3):
                            _tr(nc, pstr, f"ha_{m}_{i}",
                                hall[:, m, i * 128:(i + 1) * 128], ident[:],
                                [(hallT[:, i, m * 128:(m + 1) * 128], "v")])
                    w_rest = {}
                    for p in range(6):
                        seg = ATTN_SEG[p]
                        tiles = w_feat[p] if p < 3 else \
                            [st1a.tile([128, N], F32, name=f"w{p}_{m}")
                             for m in range(MT)]
                        if p >= 3:
                            w_rest[p] = tiles
                        for m in range(MT):
                            base, ti = (p % 2) * DS, p // 2
                            e_src = et_sb if base == 0 else et_hi
                            e_ap = e_src[base:base + DS,
                                         seg * N:(seg + 1) * N]
                            _routing(nc, rtp, psmm, f"r{p}_{m}",
                                     hallT[base:base + DS, ti,
                                           m * 128:(m + 1) * 128],
                                     e_ap, tiles[m][:])
                    if dbg:
                        nm6 = ["w_fq", "w_fk", "w_fv", "w_rq", "w_rk", "w_rv"]
                        for p in range(6):
                            td = dbg_tensor(nm6[p], [TL, N])
                            tiles = w_feat[p] if p < 3 else w_rest[p]
                            for m in range(MT):
                                nc.sync.dma_start(td[m * 128:(m + 1) * 128, :],
                                                  tiles[m][:])
                    for p, key in [(3, "rq"), (4, "rk"), (5, "rv")]:
                        for m in range(MT):
                            _tr(nc, pstr, f"wt_{p}_{m}", w_rest[p][m][:],
                                ident[:],
                                [(wtt_sb[key][:, m * 128:(m + 1) * 128], "v")])
                        nc.gpsimd.dma_start(wt_dram[key][0], wtt_sb[key][:])

                    # features (qk shared for Q and K; v)
                    _feature(nc, fp, psf, fqk_d, nxT_r,
                             [(w_feat[0], h_q), (w_feat[1], h_k)],
                             [nc.sync, nc.scalar, nc.gpsimd])
                    _feature(nc, fp, psf, fv_d, nxT_r,
                             [(w_feat[2], h_v)], [nc.sync, nc.scalar, nc.gpsimd])
                    if dbg:
                        for nm, hh in [("h_q", h_q), ("h_k", h_k), ("h_v", h_v)]:
                            td = dbg_tensor(nm, [TL, R])
                            for m in range(MT):
                                nc.sync.dma_start(td[m * 128:(m + 1) * 128, :],
                                                  hh[m][:])
                    for nm, hh in [("q", h_q), ("k", h_k), ("v", h_v)]:
                        for m in range(MT):
                            for rb in range(4):
                                _tr(nc, pstr, f"h{nm}_{m}_{rb}",
                                    hh[m][:, rb * 128:(rb + 1) * 128], ident[:],
                                    [(hT[nm][:, rb, m * 128:(m + 1) * 128], "v")])

                # restores: Q+K fused (stream r_qk once), then V
                pools = {"rchunk": rp, "wb": wbp, "gt": gtp}
                with tc.tile_pool(name="ps_y", bufs=1, space="PSUM") as psy:
                    _restore(nc, pools, rqk_d,
                             [(hT["q"][:], wt_dram["rq"][0], yT_q[:]),
                              (hT["k"][:], wt_dram["rk"][0], yT_k[:])],
                             psy, [nc.sync, nc.scalar])
                # K^T gather starts while the V restore computes
                nc.sync.dma_start(cck_in[:],
                                  yT_k[:].rearrange("p k t -> p (k t)"))
                nc.gpsimd.collective_compute(
                    "AllGather", OP.bypass,
                    ins=[cck_in[:]],
                    outs=[cck_out[:]],
                    replica_groups=[[0, 1, 2, 3], [4, 5, 6, 7]],
                )
                with tc.tile_pool(name="ps_y2", bufs=1, space="PSUM") as psy:
                    _restore(nc, pools, rv_d,
                             [(hT["v"][:], wt_dram["rv"][0], yT_v[:])],
                             psy, [nc.sync, nc.scalar])
                if dbg:
                    for nm, yy in [("yT_q", yT_q), ("yT_k", yT_k),
                                   ("yT_v", yT_v)]:
                        td = dbg_tensor(nm, [D, TL])
                        for dt in range(KT):
                            nc.gpsimd.dma_start(td[dt * 128:(dt + 1) * 128, :],
                                                yy[:, dt, :])

                with tc.tile_pool(name="ps_tr2", bufs=2, space="PSUM") as pstr2:
                    for dt in range(KT):
                        for m in range(MT):
                            _tr(nc, pstr2, f"v_{dt}_{m}",
                                yT_v[:, dt, m * 128:(m + 1) * 128], ident_b[:],
                                [(v_tok[:, m, dt * 128:(dt + 1) * 128], "s")])
                for m in range(MT):
                    nc.sync.dma_start(ccv_in[:, m * D:(m + 1) * D],
                                      v_tok[:, m, :])
            nc.gpsimd.collective_compute(
                "AllGather", OP.bypass,
                ins=[ccv_in[:]],
                outs=[ccv_out[:]],
                replica_groups=[[0, 1, 2, 3], [4, 5, 6, 7]],
            )

            # ============ stage 2: attention + W_o ============
            late_cm = tc.tile_pool(name="late", bufs=1)
            late = late_cm.__enter__()
            x2 = late.tile([128, MT, D], F32)
            ot_sb = late.tile([128, KT, TL], F32)
            with (
                tc.tile_pool(name="st2", bufs=1) as st2,
                tc.tile_pool(name="attp", bufs=3) as att,
                tc.tile_pool(name="ps_att", bufs=2, space="PSUM") as psa,
                tc.tile_pool(name="ps_ot", bufs=4, space="PSUM") as psot,
            ):
                maskT_sb = st2.tile([128, SEQ_BLOCKS, TL], F32)
                for kb in range(SEQ_BLOCKS):
                    nc.scalar.dma_start(maskT_sb[:, kb, :],
                                        maskT_d[kb * 128:(kb + 1) * 128, :])
                kt_all = st2.tile([128, 4, KT * TL], BF16)
                v_all = st2.tile([128, 4, MT * D], BF16)
                ld_engs = [nc.sync, nc.scalar, nc.gpsimd]
                for ch in range(4):
                    ld_engs[ch % 3].dma_start(
                        kt_all[:, ch, :], cck_out[ch * 128:(ch + 1) * 128, :])
                    ld_engs[(ch + 1) % 3].dma_start(
                        v_all[:, ch, :], ccv_out[ch * 128:(ch + 1) * 128, :])
                for hp in range(KT):
                    for hh in range(2):
                        pot = psot.tile([DH, TL], F32, name="pot", tag="pot")
                        nc.vector.memset(pot[:], 0.0)
                        h_idx = hp * 2 + hh
                        qt_ap = yT_q[hh * DH:(hh + 1) * DH, hp, :]
                        pss = psa.tile([1, TL], F32, name="pss", tag="pss")
                        nc.vector.memset(pss[:], 0.0)
                        for kb in range(SEQ_BLOCKS):
                            ch, m2 = kb // 2, kb % 2
                            ktap = kt_all[hh * DH:(hh + 1) * DH, ch,
                                          hp * TL + m2 * 128:
                                          hp * TL + (m2 + 1) * 128]
                            vap = v_all[:, ch,
                                        m2 * D + h_idx * DH:
                                        m2 * D + (h_idx + 1) * DH]
                            pscore = psa.tile([128, TL], F32, name="pscore",
                                              tag="pscore")
                            nc.tensor.matmul(pscore[:], ktap, qt_ap,
                                             start=True, stop=True)
                            msc = att.tile([128, TL], F32, name="msc", tag="msc")
                            nc.vector.tensor_add(msc[:], pscore[:],
                                                 maskT_sb[:, kb, :])
                            expt = att.tile([128, TL], BF16, name="expt",
                                            tag="expt")
                            nc.scalar.activation(expt[:], msc[:], ACT.Exp,
                                                 scale=0.125)
                            nc.tensor.matmul(pss[:], ones_b[:], expt[:],
                                             start=False,
                                             stop=(kb == SEQ_BLOCKS - 1))
                            nc.tensor.matmul(pot[:], vap, expt[:], start=False,
                                             stop=(kb == SEQ_BLOCKS - 1))
                        rsr = att.tile([1, TL], F32, name="rsr", tag="rsr")
                        nc.vector.reciprocal(rsr[:], pss[:])
                        rbc = att.tile([DH, TL], F32, name="rbc", tag="rbc")
                        nc.gpsimd.partition_broadcast(rbc[:], rsr[:], channels=DH)
                        otn = att.tile([DH, TL], F32, name="otn", tag="otn")
                        nc.vector.tensor_mul(otn[:], pot[:], rbc[:])
                        # SBUF->SBUF DMA can shift partitions (DVE cannot)
                        nc.sync.dma_start(ot_sb[hh * DH:(hh + 1) * DH, hp, :],
                                          otn[:])
                if dbg:
                    td = dbg_tensor("oT", [D, TL])
                    for dt in range(KT):
                        nc.sync.dma_start(td[dt * 128:(dt + 1) * 128, :],
                                          ot_sb[:, dt, :])

            with (
                tc.tile_pool(name="wop", bufs=3) as wop,
                tc.tile_pool(name="ps_mm2", bufs=2, space="PSUM") as psmm2,
            ):
                ot_r = ot_sb[:].bitcast(F32R)
                for blk in range(2):
                    wo_t = []
                    for k in range(KT):
                        wt_k = wop.tile([128, 512], F32R, name=f"wo{k}",
                                        tag="wo")
                        [nc.sync, nc.scalar][k % 2].dma_start(
                            wt_k[:], wo_d[:, k, blk * 512:(blk + 1) * 512])
                        wo_t.append(wt_k)
                    for m in range(MT):
                        px = psmm2.tile([128, 512], F32, name="px", tag="mm")
                        for k in range(KT):
                            nc.tensor.matmul(px[:],
                                             ot_r[:, k, m * 128:(m + 1) * 128],
                                             wo_t[k][:],
                                             start=(k == 0), stop=(k == KT - 1))
                        nc.vector.tensor_add(
                            x2[:, m, blk * 512:(blk + 1) * 512], px[:],
                            x_sb[:, m, blk * 512:(blk + 1) * 512])
            if dbg:
                td = dbg_tensor("x2", [TL, D])
                for m in range(MT):
                    nc.sync.dma_start(td[m * 128:(m + 1) * 128, :], x2[:, m, :])

            # ============ stage 3: knowledge circuit ============
            with (
                tc.tile_pool(name="st3", bufs=1) as st3,
                tc.tile_pool(name="fchunk2", bufs=3) as fp2,
                tc.tile_pool(name="rchunk2", bufs=3) as rp2,
                tc.tile_pool(name="wbp2", bufs=2) as wbp2,
                tc.tile_pool(name="gtp2", bufs=3) as gtp2,
            ):
                nx2T_r = st3.tile([128, KT, TL], BF16)
                h_kn = [st3.tile([128, R], F32, name=f"h_kn{m}")
                        for m in range(MT)]
                hT_kn = st3.tile([128, 4, TL], F32)
                wtt_kn = st3.tile([N, TL], F32, name="wtt_kn")
                w_kn = {}
                yT_kn = st3.tile([128, KT, TL], F32)
                with (
                    tc.tile_pool(name="st3a", bufs=1) as st3a,
                    tc.tile_pool(name="ps_tr3", bufs=2, space="PSUM") as pstr3,
                    tc.tile_pool(name="ps_mm3", bufs=2, space="PSUM") as psmm3,
                    tc.tile_pool(name="ps_feat3", bufs=2, space="PSUM") as psf3,
                ):
                    nx2 = st3a.tile([128, MT, D], F32)
                    ln2_bc = st3a.tile([128, 2, D], F32)
                    for i in range(2):
                        nc.sync.dma_start(
                            ln2_bc[:, i, :],
                            ln_d[i + 2:i + 3, :].broadcast_to([128, D]))
                    for m in range(MT):
                        _layernorm(nc, lnp, f"ln2_{m}", x2[:, m, :],
                                   ln2_bc[:, 0, :], ln2_bc[:, 1, :], nx2[:, m, :])
                    nx2T = st3a.tile([128, KT, TL], F32)
                    for m in range(MT):
                        for k in range(KT):
                            _tr(nc, pstr3, f"nx2_{m}_{k}",
                                nx2[:, m, k * 128:(k + 1) * 128], ident[:],
                                [(nx2T[:, k, m * 128:(m + 1) * 128], "v"),
                                 (nx2T_r[:, k, m * 128:(m + 1) * 128], "s")])
                    wk_sb = st3a.tile([128, KT, 2 * DS], F32)
                    nc.sync.dma_start(wk_sb[:, :, :DS], wfk_d[:])
                    nc.sync.dma_start(wk_sb[:, :, DS:], wrk_d[:])
                    hkT = st3a.tile([DS, 2, TL], F32)
                    for m in range(MT):
                        for j in range(2):
                            pk = psmm3.tile([128, DS], F32, name="pk", tag="mm")
                            for k in range(KT):
                                nc.tensor.matmul(
                                    pk[:], nx2T[:, k, m * 128:(m + 1) * 128],
                                    wk_sb[:, k, j * DS:(j + 1) * DS],
                                    start=(k == 0), stop=(k == KT - 1))
                            hk = rtp.tile([128, DS], F32, name=f"hk{m}{j}",
                                          tag="hk")
                            nc.vector.tensor_add(
                                hk[:], pk[:],
                                bias_bc[:, (6 + j) * DS:(7 + j) * DS])
                            _tr(nc, pstr3, f"hk_{m}_{j}", hk[:], ident[:],
                                [(hkT[:, j, m * 128:(m + 1) * 128], "v")])
                    for j, nm in [(0, "fkn"), (1, "rkn")]:
                        w_kn[nm] = []
                        for m in range(MT):
                            wt = st3.tile([128, N], F32, name=f"wkn{j}_{m}")
                            _routing(nc, rtp, psmm3, f"rk{j}_{m}",
                                     hkT[:, j, m * 128:(m + 1) * 128],
                                     et_sb[:, (4 + j) * N:(5 + j) * N], wt[:])
                            w_kn[nm].append(wt)
                    if dbg:
                        for nm, key in [("w_fknow", "fkn"), ("w_rknow", "rkn")]:
                            td = dbg_tensor(nm, [TL, N])
                            for m in range(MT):
                                nc.sync.dma_start(td[m * 128:(m + 1) * 128, :],
                                                  w_kn[key][m][:])
                    for m in range(MT):
                        _tr(nc, pstr3, f"wt_kn_{m}", w_kn["rkn"][m][:],
                            ident[:], [(wtt_kn[:, m * 128:(m + 1) * 128], "v")])
                    nc.gpsimd.dma_start(wt_dram["rkn"][0], wtt_kn[:])

                    _feature(nc, fp2, psf3, fkn_d, nx2T_r,
                             [(w_kn["fkn"], h_kn)],
                             [nc.sync, nc.scalar, nc.gpsimd])
                    if dbg:
                        td = dbg_tensor("h_know", [TL, R])
                        for m in range(MT):
                            nc.sync.dma_start(td[m * 128:(m + 1) * 128, :],
                                              h_kn[m][:])
                    for m in range(MT):
                        for rb in range(4):
                            _tr(nc, pstr3, f"hkn_{m}_{rb}",
                                h_kn[m][:, rb * 128:(rb + 1) * 128], ident[:],
                                [(hT_kn[:, rb, m * 128:(m + 1) * 128], "v")])

                pools3 = {"rchunk": rp2, "wb": wbp2, "gt": gtp2}
                with tc.tile_pool(name="ps_y3", bufs=1, space="PSUM") as psy3:
                    _restore(nc, pools3, rkn_d,
                             [(hT_kn[:], wt_dram["rkn"][0], yT_kn[:])],
                             psy3, [nc.sync, nc.scalar])

                out_sb = st3.tile([128, MT, D], F32)
                with tc.tile_pool(name="ps_fin", bufs=2, space="PSUM") as psfin:
                    for dt in range(KT):
                        for m in range(MT):
                            p = psfin.tile([128, 128], F32,
                                           name=f"fin_{dt}_{m}", tag="fin")
                            nc.tensor.transpose(
                                p[:], yT_kn[:, dt, m * 128:(m + 1) * 128],
                                ident[:])
                            nc.vector.tensor_add(
                                out_sb[:, m, dt * 128:(dt + 1) * 128], p[:],
                                x2[:, m, dt * 128:(dt + 1) * 128])
                for m in range(MT):
                    nc.sync.dma_start(y_d[m * 128:(m + 1) * 128, :],
                                      out_sb[:, m, :])
            late_cm.__exit__(None, None, None)

    nc.compile()
    return nc, dbg_t


def prep_inputs(inputs):
    f32 = np.float32
    bf16 = mybir.dt.np(BF16)
    x = np.ascontiguousarray(np.asarray(inputs["x"], f32).reshape(T, D))
    ne = np.asarray(inputs["neuron_emb"], f32)
    emb = ne / (np.linalg.norm(ne, axis=-1, keepdims=True) + 1e-8)

    def f_layout(f):
        f = np.asarray(f, f32)
        return np.ascontiguousarray(
            f.reshape(N, KT, 128, R).transpose(2, 0, 1, 3)
            .reshape(128, N, KT * R).astype(bf16))

    def r_layout(r):
        r = np.asarray(r, f32).reshape(N * R, D)
        return np.ascontiguousarray(
            r.reshape(NRT, 128, D).transpose(1, 0, 2).astype(bf16))

    def w_layout(w):
        w = np.asarray(w, f32)
        return np.ascontiguousarray(
            w.reshape(KT, 128, w.shape[-1]).transpose(1, 0, 2))

    shared = {
        "wall": w_layout(inputs["W_all"]),
        "wo": w_layout(inputs["W_o"]),
        "wfk": w_layout(inputs["W_fk"]),
        "wrk": w_layout(inputs["W_rk"]),
        "et": np.ascontiguousarray(emb.T),
        "fqk": f_layout(inputs["f_qk"]),
        "fv": f_layout(inputs["f_v"]),
        "fkn": f_layout(inputs["f_know"]),
        "rqk": r_layout(inputs["r_qk"]),
        "rv": r_layout(inputs["r_v"]),
        "rkn": r_layout(inputs["r_know"]),
        "lnrows": np.ascontiguousarray(
            np.stack([np.asarray(inputs[k], f32)
                      for k in ("ln1_s", "ln1_b", "ln2_s", "ln2_b")])),
        "biasrow": np.ascontiguousarray(
            np.concatenate([np.asarray(inputs["b_all"], f32),
                            np.asarray(inputs["b_fk"], f32),
                            np.asarray(inputs["b_rk"], f32)])[None, :]),
    }
    per_core = []
    k_idx = np.arange(S)[:, None]
    for c in range(NCORES):
        ci = c % (S // TL)
        q_idx = ci * TL + np.arange(TL)[None, :]
        maskT = np.where(k_idx <= q_idx, 0.0, NEG).astype(f32)
        per_core.append({
            "x": np.ascontiguousarray(x[c * TL:(c + 1) * TL]),
            "maskT": np.ascontiguousarray(maskT),
            **shared,
        })
    return per_core


def kernel(**inputs):
    global _PROG
    if _PROG is None:
        _PROG = build(dbg=False)
    nc, _ = _PROG
    per_core = prep_inputs(inputs)
    res = run_bass_kernel_spmd(nc, per_core, core_ids=list(range(NCORES)))
    y = np.concatenate([res.results[c]["y"] for c in range(NCORES)], axis=0)
    return y.reshape(B, S, D).astype(np.float32)



# revision 56
# speedup vs baseline: 1.8933x; 1.0469x over previous
"""Trainium2 Bass kernel for the moe_routing problem (nn_DAWN_69904887709893).

Token-parallel across 8 NeuronCores (256 tokens/core), neuron pools replicated.
Heavy einsums (feature/restore/attention) run in fp32r on the PE; every matmul
feeding router logits (W_all, logit projections, W_fk/W_rk, W_o) runs in plain
fp32 so top-k selections match the fp32 reference. K^T and token-major V are
exchanged with a 4-rank AllGather per sequence group; causality comes from a
host-provided additive mask so the SPMD program is identical on every core.

PSUM accumulators that pack two regions per bank are pre-zeroed with memset and
use start=False matmuls throughout: correct regardless of has_written state and
immune to instruction reordering (a start=True clears the whole bank's bits).
"""
import sys

sys.path.insert(0, "/opt/trn_rl_repo")
import numpy as np
import concourse.bass as bass
import concourse.bacc as bacc
import concourse.mybir as mybir
import concourse.tile as tile
from concourse.bass_utils import run_bass_kernel_spmd
from concourse.masks import make_identity

F32 = mybir.dt.float32
F32R = mybir.dt.float32r
BF16 = mybir.dt.bfloat16
AX = mybir.AxisListType.X
OP = mybir.AluOpType
ACT = mybir.ActivationFunctionType

NCORES = 8
B, S, D, R, N, DS, TOPK, H = 2, 1024, 1024, 512, 32, 64, 4, 16
T = B * S
TL = T // NCORES          # tokens per core (256)
MT = TL // 128            # token tiles per core (2)
KT = D // 128             # contraction tiles over D (8)
DH = D // H               # head dim (64)
NRT = (N * R) // 128      # contraction tiles over N*R (128)
SEQ_BLOCKS = S // 128     # k blocks per sequence (8)
NEG = -1.0e30

# emb segment used by each of the 6 attention routings (fq, fk, fv, rq, rk, rv)
ATTN_SEG = [0, 0, 1, 2, 2, 3]

_PROG = None


def _routing(nc, rt, psmm, name, lhsT_ap, e_ap, wout):
    """softmax over 32 logits + top-4 sparsify + renormalize -> wout [128,32] f32.
    lhsT_ap: [64, 128] fp32 (h-segment transposed), e_ap: [64, 32] fp32."""
    lg = psmm.tile([128, N], F32, name=f"lg_{name}", tag="mm")
    nc.tensor.matmul(lg[:], lhsT_ap, e_ap, start=True, stop=True)
    mx = rt.tile([128, 1], F32, name=f"mx_{name}", tag="mx")
    nc.vector.tensor_reduce(mx[:], lg[:], AX, OP.max)
    nmx = rt.tile([128, 1], F32, name=f"nmx_{name}", tag="nmx")
    nc.scalar.mul(nmx[:], mx[:], -1.0)
    ex = rt.tile([128, N], F32, name=f"ex_{name}", tag="ex")
    ssum = rt.tile([128, 1], F32, name=f"ssum_{name}", tag="ssum")
    nc.scalar.activation(ex[:], lg[:], ACT.Exp, bias=nmx[:], scale=1.0,
                         accum_out=ssum[:])
    rs = rt.tile([128, 1], F32, name=f"rs_{name}", tag="rs")
    nc.vector.reciprocal(rs[:], ssum[:])
    sm = rt.tile([128, N], F32, name=f"sm_{name}", tag="sm")
    nc.vector.tensor_scalar_mul(sm[:], ex[:], rs[:])
    top8 = rt.tile([128, 8], F32, name=f"top8_{name}", tag="top8")
    nc.vector.max(top8[:], sm[:])
    ge = rt.tile([128, N], F32, name=f"ge_{name}", tag="ge")
    nc.vector.tensor_scalar(ge[:], sm[:], top8[:, 3:4], None, OP.is_ge)
    sp = rt.tile([128, N], F32, name=f"sp_{name}", tag="sp")
    nc.vector.tensor_mul(sp[:], sm[:], ge[:])
    s2 = rt.tile([128, 1], F32, name=f"s2_{name}", tag="s2")
    nc.vector.tensor_reduce(s2[:], sp[:], AX, OP.add)
    s2e = rt.tile([128, 1], F32, name=f"s2e_{name}", tag="s2e")
    nc.vector.tensor_scalar_add(s2e[:], s2[:], 1e-8)
    rs2 = rt.tile([128, 1], F32, name=f"rs2_{name}", tag="rs2")
    nc.vector.reciprocal(rs2[:], s2e[:])
    nc.vector.tensor_scalar_mul(wout, sp[:], rs2[:])


def _layernorm(nc, lnp, name, x_ap, s_bc, b_bc, out_ap):
    """LN over the free dim (D). x_ap/out_ap [128, D] f32; s_bc/b_bc [128, D]."""
    mu = lnp.tile([128, 1], F32, name=f"mu_{name}", tag="mu")
    nc.vector.tensor_reduce(mu[:], x_ap, AX, OP.add)
    nc.scalar.mul(mu[:], mu[:], 1.0 / D)
    xc = lnp.tile([128, D], F32, name=f"xc_{name}", tag="xc")
    nc.vector.tensor_scalar_sub(xc[:], x_ap, mu[:])
    sq = lnp.tile([128, D], F32, name=f"sq_{name}", tag="sq")
    vs = lnp.tile([128, 1], F32, name=f"vs_{name}", tag="vs")
    nc.scalar.activation(sq[:], xc[:], ACT.Square, accum_out=vs[:])
    nc.scalar.activation(vs[:], vs[:], ACT.Copy, scale=1.0 / D, bias=1e-6)
    rv = lnp.tile([128, 1], F32, name=f"rv_{name}", tag="rv")
    nc.vector.reciprocal(rv[:], vs[:])
    rstd = lnp.tile([128, 1], F32, name=f"rstd_{name}", tag="rstd")
    nc.scalar.activation(rstd[:], rv[:], ACT.Sqrt)
    nc.vector.scalar_tensor_tensor(out_ap, xc[:], rstd[:], s_bc, OP.mult, OP.mult)
    nc.vector.tensor_add(out_ap, out_ap, b_bc)


def _tr(nc, pstr, name, src_ap, ident, outs):
    """PE-transpose a [128, <=128] block; copy the psum into each (ap, engine)."""
    p = pstr.tile([src_ap.shape[-1], 128], src_ap.dtype, name=f"tr_{name}",
                  tag="tr")
    nc.tensor.transpose(p[:], src_ap, ident)
    for ap, eng in outs:
        if eng == "v":
            nc.vector.tensor_copy(ap, p[:, :ap.shape[-1]])
        else:
            nc.scalar.copy(ap, p[:, :ap.shape[-1]])


def _feature(nc, fp, psf, fdram, nxT_r, routes, engs):
    """h[m] accumulators += w[:,n] * (nx @ f_n) for all 32 neurons.
    routes: list of (w_tiles_per_m, hacc_per_m). F is streamed in half-neuron
    chunks (4 k-tiles each) to halve SBUF residency."""
    HKT = KT // 2
    for n in range(N):
        pfs = [psf.tile([128, R], F32, name=f"pf{m}", tag=f"pf{m}")
               for m in range(MT)]
        for half in range(2):
            fc = fp.tile([128, HKT * R], fdram.dtype, name="fc", tag="fc")
            engs[(2 * n + half) % len(engs)].dma_start(
                fc[:], fdram[:, n, half * HKT * R:(half + 1) * HKT * R])
            for m in range(MT):
                for kk in range(HKT):
                    k = half * HKT + kk
                    nc.tensor.matmul(pfs[m][:],
                                     nxT_r[:, k, m * 128:(m + 1) * 128],
                                     fc[:, kk * R:(kk + 1) * R],
                                     start=(k == 0), stop=(k == KT - 1))
        for m in range(MT):
            for wt, hacc in routes:
                w_ap = wt[m][:, n:n + 1]
                if n == 0:
                    nc.vector.tensor_scalar(hacc[m][:], pfs[m][:], w_ap, None,
                                            OP.mult)
                else:
                    nc.vector.scalar_tensor_tensor(hacc[m][:], pfs[m][:], w_ap,
                                                   hacc[m][:], OP.mult, OP.add)


def _restore(nc, pools, rdram, routes, psy, engs):
    """Transposed-output restores sharing one streamed r matrix.
    routes: list of (hT_tile, w_sb [N, TL] sbuf tile, yT_out_ap).
    Packs len(routes) [128, TL] accumulators per PSUM bank; all matmuls use
    start=False on memset-zeroed banks."""
    nr = len(routes)
    assert nr in (1, 2)
    per_bank = 2
    n_banks = (KT * nr + per_bank - 1) // per_bank
    pys = [psy.tile([128, per_bank, TL], F32, name=f"ry{i}", tag=f"ry{i}")
           for i in range(n_banks)]
    for py in pys:
        nc.vector.memset(py[:], 0.0)

    def acc_ap(ri, dt):
        flat = dt * nr + ri
        return pys[flat // per_bank][:, flat % per_bank, :]

    rp, wbp, gtp = pools["rchunk"], pools["wb"], pools["gt"]
    for kt in range(NRT):
        n, rb = kt // 4, kt % 4
        wbs = []
        for ri, (hT, wtd, _) in enumerate(routes):
            if rb == 0:
                wb = wbp.tile([128, TL], F32, name=f"wb{ri}", tag=f"wb{ri}")
                nc.gpsimd.dma_start(wb[:],
                                    wtd[n:n + 1, :].broadcast_to([128, TL]))
                pools[f"_wb{ri}"] = wb
            wbs.append(pools[f"_wb{ri}"])
        if kt % 2 == 0:
            rc = rp.tile([128, 2, D], rdram.dtype, name="rc", tag="rc")
            engs[(kt // 2) % len(engs)].dma_start(rc[:], rdram[:, kt:kt + 2, :])
            pools["_rc"] = rc
        rc = pools["_rc"]
        for ri, (hT, _, _) in enumerate(routes):
            gt = gtp.tile([128, TL], rdram.dtype, name=f"gt{ri}", tag=f"gt{ri}")
            nc.vector.tensor_mul(gt[:], hT[:, rb, :], wbs[ri][:])
            for dt in range(KT):
                nc.tensor.matmul(acc_ap(ri, dt),
                                 rc[:, kt % 2, dt * 128:(dt + 1) * 128],
                                 gt[:], start=False, stop=(kt == NRT - 1))
    for ri, (_, _, yT_out) in enumerate(routes):
        for dt in range(KT):
            eng = nc.scalar if (dt + ri) % 2 == 0 else nc.vector
            if eng is nc.scalar:
                nc.scalar.copy(yT_out[:, dt, :], acc_ap(ri, dt))
            else:
                nc.vector.tensor_copy(yT_out[:, dt, :], acc_ap(ri, dt))


def build(dbg=False):
    nc = bacc.Bacc("TRN2", target_bir_lowering=False, debug=False,
                   num_devices=NCORES)

    x_d = nc.dram_tensor("x", [TL, D], F32, kind="ExternalInput")
    maskT_d = nc.dram_tensor("maskT", [S, TL], F32, kind="ExternalInput")
    wall_d = nc.dram_tensor("wall", [128, KT, 6 * DS], F32, kind="ExternalInput")
    wo_d = nc.dram_tensor("wo", [128, KT, D], F32R, kind="ExternalInput")
    wfk_d = nc.dram_tensor("wfk", [128, KT, DS], F32, kind="ExternalInput")
    wrk_d = nc.dram_tensor("wrk", [128, KT, DS], F32, kind="ExternalInput")
    et_d = nc.dram_tensor("et", [DS, 6 * N], F32, kind="ExternalInput")
    fqk_d = nc.dram_tensor("fqk", [128, N, KT * R], BF16, kind="ExternalInput")
    fv_d = nc.dram_tensor("fv", [128, N, KT * R], BF16, kind="ExternalInput")
    fkn_d = nc.dram_tensor("fkn", [128, N, KT * R], BF16, kind="ExternalInput")
    rqk_d = nc.dram_tensor("rqk", [128, NRT, D], BF16, kind="ExternalInput")
    rv_d = nc.dram_tensor("rv", [128, NRT, D], BF16, kind="ExternalInput")
    rkn_d = nc.dram_tensor("rkn", [128, NRT, D], BF16, kind="ExternalInput")
    ln_d = nc.dram_tensor("lnrows", [4, D], F32, kind="ExternalInput")
    bias_d = nc.dram_tensor("biasrow", [1, 8 * DS], F32, kind="ExternalInput")
    y_d = nc.dram_tensor("y", [TL, D], F32, kind="ExternalOutput")

    dbg_t = {}

    def dbg_tensor(name, shape):
        dbg_t[name] = nc.dram_tensor("dbg_" + name, shape, F32,
                                     kind="ExternalOutput")
        return dbg_t[name]

    with tile.TileContext(nc) as tc:
        with (
            tc.tile_pool(name="perm", bufs=1) as perm,
            tc.tile_pool(name="dramp", bufs=1, space="DRAM") as dramp,
            tc.tile_pool(name="lnp", bufs=1) as lnp,
            tc.tile_pool(name="rtp", bufs=2) as rtp,
        ):
            # collective bounce buffers (K^T gathered first, then token-major V)
            cck_in = dramp.tile([128, KT * TL], BF16, name="cck_in")
            cck_out = dramp.tile([4 * 128, KT * TL], BF16, name="cck_out")
            ccv_in = dramp.tile([128, MT * D], BF16, name="ccv_in")
            ccv_out = dramp.tile([4 * 128, MT * D], BF16, name="ccv_out")
            # restore-route w rows, bounced through DRAM into one partition
            wt_dram = {k: dramp.tile([1, N, TL], F32, name=f"wtd_{k}")
                       for k in ("rq", "rk", "rv", "rkn")}

            ident = perm.tile([128, 128], F32)
            make_identity(nc, ident[:])
            ones_f = perm.tile([128, 1], F32)
            nc.gpsimd.memset(ones_f[:], 1.0)
            ones_b = perm.tile([128, 1], BF16)
            nc.vector.tensor_copy(ones_b[:], ones_f[:])
            bias_bc = perm.tile([128, 8 * DS], F32)
            nc.sync.dma_start(bias_bc[:], bias_d[0:1, :].broadcast_to([128, 8 * DS]))
            et_sb = perm.tile([DS, 6 * N], F32)
            nc.sync.dma_start(et_sb[:], et_d[:])
            # copy at partition base 64 for routings whose h-segment sits in
            # the upper half of a transposed tile (matmul requires equal bases)
            et_hi = perm.tile([128, 6 * N], F32)
            nc.sync.dma_start(et_hi[DS:2 * DS, :], et_d[:])
            x_sb = perm.tile([128, MT, D], F32)
            for m in range(MT):
                nc.sync.dma_start(x_sb[:, m, :], x_d[m * 128:(m + 1) * 128, :])
            maskT_sb = perm.tile([128, SEQ_BLOCKS, TL], F32)
            for kb in range(SEQ_BLOCKS):
                nc.scalar.dma_start(maskT_sb[:, kb, :],
                                    maskT_d[kb * 128:(kb + 1) * 128, :])
            yT_q = perm.tile([128, KT, TL], BF16)

            # ============ stage 1: LN1 + routing + features + restores ========
            with (
                tc.tile_pool(name="st1", bufs=1) as st1,
                tc.tile_pool(name="fchunk", bufs=4) as fp,
                tc.tile_pool(name="rchunk", bufs=4) as rp,
                tc.tile_pool(name="wbp", bufs=2) as wbp,
                tc.tile_pool(name="gtp", bufs=3) as gtp,
            ):
                nxT_r = st1.tile([128, KT, TL], BF16)
                h_q = [st1.tile([128, R], F32, name=f"h_q{m}") for m in range(MT)]
                h_k = [st1.tile([128, R], F32, name=f"h_k{m}") for m in range(MT)]
                h_v = [st1.tile([128, R], F32, name=f"h_v{m}") for m in range(MT)]
                hT = {k: st1.tile([128, 4, TL], F32, name=f"hT_{k}")
                      for k in ("q", "k", "v")}
                w_feat = {p: [st1.tile([128, N], F32, name=f"w{p}_{m}")
                              for m in range(MT)] for p in range(3)}
                wtt_sb = {k: st1.tile([N, TL], F32, name=f"wtt_{k}")
                          for k in ("rq", "rk", "rv")}
                yT_k = st1.tile([128, KT, TL], BF16)
                v_tok = st1.tile([128, MT, D], BF16)

                with (
                    tc.tile_pool(name="st1a", bufs=1) as st1a,
                    tc.tile_pool(name="ps_tr", bufs=2, space="PSUM") as pstr,
                    tc.tile_pool(name="ps_mm", bufs=2, space="PSUM") as psmm,
                    tc.tile_pool(name="ps_feat", bufs=2, space="PSUM") as psf,
                    tc.tile_pool(name="wallp", bufs=2) as wallp,
                ):
                    nxT = st1a.tile([128, KT, TL], F32)
                    nx = st1a.tile([128, MT, D], F32)
                    ln1_bc = st1a.tile([128, 2, D], F32)
                    for i in range(2):
                        nc.gpsimd.dma_start(ln1_bc[:, i, :],
                                            ln_d[i:i + 1, :]
                                            .broadcast_to([128, D]))
                    for m in range(MT):
                        _layernorm(nc, lnp, f"ln1_{m}", x_sb[:, m, :],
                                   ln1_bc[:, 0, :], ln1_bc[:, 1, :], nx[:, m, :])
                    if dbg:
                        td = dbg_tensor("nx", [TL, D])
                        for m in range(MT):
                            nc.sync.dma_start(td[m * 128:(m + 1) * 128, :],
                                              nx[:, m, :])
                    for m in range(MT):
                        for k in range(KT):
                            _tr(nc, pstr, f"nx_{m}_{k}",
                                nx[:, m, k * 128:(k + 1) * 128], ident[:],
                                [(nxT[:, k, m * 128:(m + 1) * 128], "v"),
                                 (nxT_r[:, k, m * 128:(m + 1) * 128], "s")])

                    hall = st1a.tile([128, MT, 6 * DS], F32)
                    phs = [psmm.tile([128, 6 * DS], F32, name=f"ph{m}",
                                     tag="mm") for m in range(MT)]
                    for k in range(KT):
                        wt_k = wallp.tile([128, 6 * DS], F32, name="wal",
                                          tag="wal")
                        [nc.sync, nc.scalar][k % 2].dma_start(
                            wt_k[:], wall_d[:, k, :])
                        for m in range(MT):
                            nc.tensor.matmul(phs[m][:],
                                             nxT[:, k, m * 128:(m + 1) * 128],
                                             wt_k[:],
                                             start=(k == 0), stop=(k == KT - 1))
                    for m in range(MT):
                        nc.vector.tensor_add(hall[:, m, :], phs[m][:],
                                             bias_bc[:, :6 * DS])
                    hallT = st1a.tile([128, 3, TL], F32)
                    for m in range(MT):
                        for i in range(3):
                            _tr(nc, pstr, f"ha_{m}_{i}",
                                hall[:, m, i * 128:(i + 1) * 128], ident[:],
                                [(hallT[:, i, m * 128:(m + 1) * 128], "v")])
                    w_rest = {}
                    for p in range(6):
                        seg = ATTN_SEG[p]
                        tiles = w_feat[p] if p < 3 else \
                            [st1a.tile([128, N], F32, name=f"w{p}_{m}")
                             for m in range(MT)]
                        if p >= 3:
                            w_rest[p] = tiles
                        for m in range(MT):
                            base, ti = (p % 2) * DS, p // 2
                            e_src = et_sb if base == 0 else et_hi
                            e_ap = e_src[base:base + DS,
                                         seg * N:(seg + 1) * N]
                            _routing(nc, rtp, psmm, f"r{p}_{m}",
                                     hallT[base:base + DS, ti,
                                           m * 128:(m + 1) * 128],
                                     e_ap, tiles[m][:])
                    if dbg:
                        nm6 = ["w_fq", "w_fk", "w_fv", "w_rq", "w_rk", "w_rv"]
                        for p in range(6):
                            td = dbg_tensor(nm6[p], [TL, N])
                            tiles = w_feat[p] if p < 3 else w_rest[p]
                            for m in range(MT):
                                nc.sync.dma_start(td[m * 128:(m + 1) * 128, :],
                                                  tiles[m][:])
                    for p, key in [(3, "rq"), (4, "rk"), (5, "rv")]:
                        for m in range(MT):
                            _tr(nc, pstr, f"wt_{p}_{m}", w_rest[p][m][:],
                                ident[:],
                                [(wtt_sb[key][:, m * 128:(m + 1) * 128], "v")])
                        nc.gpsimd.dma_start(wt_dram[key][0], wtt_sb[key][:])

                    # features (qk shared for Q and K; v)
                    _feature(nc, fp, psf, fqk_d, nxT_r,
                             [(w_feat[0], h_q), (w_feat[1], h_k)],
                             [nc.sync, nc.scalar, nc.gpsimd])
                    _feature(nc, fp, psf, fv_d, nxT_r,
                             [(w_feat[2], h_v)], [nc.sync, nc.scalar, nc.gpsimd])
                    if dbg:
                        for nm, hh in [("h_q", h_q), ("h_k", h_k), ("h_v", h_v)]:
                            td = dbg_tensor(nm, [TL, R])
                            for m in range(MT):
                                nc.sync.dma_start(td[m * 128:(m + 1) * 128, :],
                                                  hh[m][:])
                    for nm, hh in [("q", h_q), ("k", h_k), ("v", h_v)]:
                        for m in range(MT):
                            for rb in range(4):
                                _tr(nc, pstr, f"h{nm}_{m}_{rb}",
                                    hh[m][:, rb * 128:(rb + 1) * 128], ident[:],
                                    [(hT[nm][:, rb, m * 128:(m + 1) * 128], "v")])

                # restores: Q+K fused (stream r_qk once), then V
                pools = {"rchunk": rp, "wb": wbp, "gt": gtp}
                with tc.tile_pool(name="ps_y", bufs=1, space="PSUM") as psy:
                    _restore(nc, pools, rqk_d,
                             [(hT["q"][:], wt_dram["rq"][0], yT_q[:]),
                              (hT["k"][:], wt_dram["rk"][0], yT_k[:])],
                             psy, [nc.sync, nc.scalar])
                # K^T gather starts while the V restore computes
                nc.sync.dma_start(cck_in[:],
                                  yT_k[:].rearrange("p k t -> p (k t)"))
                nc.gpsimd.collective_compute(
                    "AllGather", OP.bypass,
                    ins=[cck_in[:]],
                    outs=[cck_out[:]],
                    replica_groups=[[0, 1, 2, 3], [4, 5, 6, 7]],
                )
                # V restore in token-major orientation: psum accumulators are
                # [tok, d-half]; v_tok comes straight out of PSUM (no
                # transposes on the V collective's critical path).
                with tc.tile_pool(name="ps_y2", bufs=1, space="PSUM") as psy:
                    pvs = [[psy.tile([128, 512], F32, name=f"pv{m}{db}",
                                     tag=f"pv{m}{db}") for db in range(2)]
                           for m in range(MT)]
                    for m in range(MT):
                        for db in range(2):
                            nc.vector.memset(pvs[m][db][:], 0.0)
                    for kt in range(NRT):
                        n, rb = kt // 4, kt % 4
                        if rb == 0:
                            wbv = wbp.tile([128, TL], F32, name="wbv",
                                           tag="wb0")
                            nc.gpsimd.dma_start(
                                wbv[:], wt_dram["rv"][0][n:n + 1, :]
                                .broadcast_to([128, TL]))
                            pools["_wbv"] = wbv
                        if kt % 2 == 0:
                            rcv = rp.tile([128, 2, D], rv_d.dtype, name="rcv",
                                          tag="rc")
                            [nc.sync, nc.scalar][(kt // 2) % 2].dma_start(
                                rcv[:], rv_d[:, kt:kt + 2, :])
                            pools["_rcv"] = rcv
                        rcv = pools["_rcv"]
                        gtv = gtp.tile([128, TL], rv_d.dtype, name="gtv",
                                       tag="gt0")
                        nc.vector.tensor_mul(gtv[:], hT["v"][:, rb, :],
                                             pools["_wbv"][:])
                        for m in range(MT):
                            for db in range(2):
                                nc.tensor.matmul(
                                    pvs[m][db][:],
                                    gtv[:, m * 128:(m + 1) * 128],
                                    rcv[:, kt % 2, db * 512:(db + 1) * 512],
                                    start=False, stop=(kt == NRT - 1))
                    for m in range(MT):
                        for db in range(2):
                            dst = v_tok[:, m, db * 512:(db + 1) * 512]
                            if (m + db) % 2 == 0:
                                nc.scalar.copy(dst, pvs[m][db][:])
                            else:
                                nc.vector.tensor_copy(dst, pvs[m][db][:])
                for m in range(MT):
                    nc.sync.dma_start(ccv_in[:, m * D:(m + 1) * D],
                                      v_tok[:, m, :])
            nc.gpsimd.collective_compute(
                "AllGather", OP.bypass,
                ins=[ccv_in[:]],
                outs=[ccv_out[:]],
                replica_groups=[[0, 1, 2, 3], [4, 5, 6, 7]],
            )

            # ============ stage 2: attention + W_o ============
            late_cm = tc.tile_pool(name="late", bufs=1)
            late = late_cm.__enter__()
            x2 = late.tile([128, MT, D], F32)
            ot_sb = late.tile([128, KT, TL], F32)
            with (
                tc.tile_pool(name="st2", bufs=1) as st2,
                tc.tile_pool(name="attp", bufs=3) as att,
                tc.tile_pool(name="ps_att", bufs=2, space="PSUM") as psa,
                tc.tile_pool(name="ps_ot", bufs=4, space="PSUM") as psot,
            ):
                # phase A: scores + exp for all heads (K only) — the V
                # AllGather and v_all loads hide under this pass.
                expt_all = st2.tile([128, 2 * KT, SEQ_BLOCKS, TL], BF16)
                with tc.tile_pool(name="ktp", bufs=1) as ktp:
                    kt_all = ktp.tile([128, 4, KT * TL], BF16)
                    ld_engs = [nc.sync, nc.scalar, nc.gpsimd]
                    for ch in range(4):
                        ld_engs[ch % 3].dma_start(
                            kt_all[:, ch, :],
                            cck_out[ch * 128:(ch + 1) * 128, :])
                    for hp in range(KT):
                        for hh in range(2):
                            h_idx = hp * 2 + hh
                            qt_ap = yT_q[hh * DH:(hh + 1) * DH, hp, :]
                            for kb in range(SEQ_BLOCKS):
                                ch, m2 = kb // 2, kb % 2
                                ktap = kt_all[hh * DH:(hh + 1) * DH, ch,
                                              hp * TL + m2 * 128:
                                              hp * TL + (m2 + 1) * 128]
                                pscore = psa.tile([128, TL], F32,
                                                  name="pscore", tag="pscore")
                                nc.tensor.matmul(pscore[:], ktap, qt_ap,
                                                 start=True, stop=True)
                                msc = att.tile([128, TL], F32, name="msc",
                                               tag="msc")
                                nc.vector.tensor_add(msc[:], pscore[:],
                                                     maskT_sb[:, kb, :])
                                nc.scalar.activation(
                                    expt_all[:, h_idx, kb, :], msc[:],
                                    ACT.Exp, scale=0.125)
                # phase B: softmax denominators + AV
                v_all = st2.tile([128, 4, MT * D], BF16)
                ld_engs = [nc.sync, nc.scalar, nc.gpsimd]
                for ch in range(4):
                    ld_engs[ch % 3].dma_start(
                        v_all[:, ch, :], ccv_out[ch * 128:(ch + 1) * 128, :])
                for hp in range(KT):
                    for hh in range(2):
                        h_idx = hp * 2 + hh
                        pot = psot.tile([DH, TL], F32, name="pot", tag="pot")
                        nc.vector.memset(pot[:], 0.0)
                        pss = psa.tile([1, TL], F32, name="pss", tag="pss")
                        nc.vector.memset(pss[:], 0.0)
                        for kb in range(SEQ_BLOCKS):
                            ch, m2 = kb // 2, kb % 2
                            vap = v_all[:, ch,
                                        m2 * D + h_idx * DH:
                                        m2 * D + (h_idx + 1) * DH]
                            nc.tensor.matmul(pss[:], ones_b[:],
                                             expt_all[:, h_idx, kb, :],
                                             start=False,
                                             stop=(kb == SEQ_BLOCKS - 1))
                            nc.tensor.matmul(pot[:], vap,
                                             expt_all[:, h_idx, kb, :],
                                             start=False,
                                             stop=(kb == SEQ_BLOCKS - 1))
                        rsr = att.tile([1, TL], F32, name="rsr", tag="rsr")
                        nc.vector.reciprocal(rsr[:], pss[:])
                        rbc = att.tile([DH, TL], F32, name="rbc", tag="rbc")
                        nc.gpsimd.partition_broadcast(rbc[:], rsr[:],
                                                      channels=DH)
                        otn = att.tile([DH, TL], F32, name="otn", tag="otn")
                        nc.vector.tensor_mul(otn[:], pot[:], rbc[:])
                        # SBUF->SBUF DMA can shift partitions (DVE cannot)
                        nc.sync.dma_start(ot_sb[hh * DH:(hh + 1) * DH, hp, :],
                                          otn[:])

            with (
                tc.tile_pool(name="wop", bufs=3) as wop,
                tc.tile_pool(name="ps_mm2", bufs=2, space="PSUM") as psmm2,
            ):
                ot_r = ot_sb[:].bitcast(F32R)
                for blk in range(2):
                    wo_t = []
                    for k in range(KT):
                        wt_k = wop.tile([128, 512], F32R, name=f"wo{k}",
                                        tag="wo")
                        [nc.sync, nc.scalar][k % 2].dma_start(
                            wt_k[:], wo_d[:, k, blk * 512:(blk + 1) * 512])
                        wo_t.append(wt_k)
                    for m in range(MT):
                        px = psmm2.tile([128, 512], F32, name="px", tag="mm")
                        for k in range(KT):
                            nc.tensor.matmul(px[:],
                                             ot_r[:, k, m * 128:(m + 1) * 128],
                                             wo_t[k][:],
                                             start=(k == 0), stop=(k == KT - 1))
                        nc.vector.tensor_add(
                            x2[:, m, blk * 512:(blk + 1) * 512], px[:],
                            x_sb[:, m, blk * 512:(blk + 1) * 512])
            if dbg:
                td = dbg_tensor("x2", [TL, D])
                for m in range(MT):
                    nc.sync.dma_start(td[m * 128:(m + 1) * 128, :], x2[:, m, :])

            # ============ stage 3: knowledge circuit ============
            with (
                tc.tile_pool(name="st3", bufs=1) as st3,
                tc.tile_pool(name="fchunk2", bufs=3) as fp2,
                tc.tile_pool(name="rchunk2", bufs=3) as rp2,
                tc.tile_pool(name="wbp2", bufs=2) as wbp2,
                tc.tile_pool(name="gtp2", bufs=3) as gtp2,
            ):
                nx2T_r = st3.tile([128, KT, TL], BF16)
                h_kn = [st3.tile([128, R], F32, name=f"h_kn{m}")
                        for m in range(MT)]
                hT_kn = st3.tile([128, 4, TL], F32)
                wtt_kn = st3.tile([N, TL], F32, name="wtt_kn")
                w_kn = {}
                yT_kn = st3.tile([128, KT, TL], F32)
                with (
                    tc.tile_pool(name="st3a", bufs=1) as st3a,
                    tc.tile_pool(name="ps_tr3", bufs=2, space="PSUM") as pstr3,
                    tc.tile_pool(name="ps_mm3", bufs=2, space="PSUM") as psmm3,
                    tc.tile_pool(name="ps_feat3", bufs=2, space="PSUM") as psf3,
                ):
                    nx2 = st3a.tile([128, MT, D], F32)
                    ln2_bc = st3a.tile([128, 2, D], F32)
                    for i in range(2):
                        nc.sync.dma_start(
                            ln2_bc[:, i, :],
                            ln_d[i + 2:i + 3, :].broadcast_to([128, D]))
                    for m in range(MT):
                        _layernorm(nc, lnp, f"ln2_{m}", x2[:, m, :],
                                   ln2_bc[:, 0, :], ln2_bc[:, 1, :], nx2[:, m, :])
                    nx2T = st3a.tile([128, KT, TL], F32)
                    for m in range(MT):
                        for k in range(KT):
                            _tr(nc, pstr3, f"nx2_{m}_{k}",
                                nx2[:, m, k * 128:(k + 1) * 128], ident[:],
                                [(nx2T[:, k, m * 128:(m + 1) * 128], "v"),
                                 (nx2T_r[:, k, m * 128:(m + 1) * 128], "s")])
                    wk_sb = st3a.tile([128, KT, 2 * DS], F32)
                    nc.sync.dma_start(wk_sb[:, :, :DS], wfk_d[:])
                    nc.sync.dma_start(wk_sb[:, :, DS:], wrk_d[:])
                    hkT = st3a.tile([DS, 2, TL], F32)
                    for m in range(MT):
                        for j in range(2):
                            pk = psmm3.tile([128, DS], F32, name="pk", tag="mm")
                            for k in range(KT):
                                nc.tensor.matmul(
                                    pk[:], nx2T[:, k, m * 128:(m + 1) * 128],
                                    wk_sb[:, k, j * DS:(j + 1) * DS],
                                    start=(k == 0), stop=(k == KT - 1))
                            hk = rtp.tile([128, DS], F32, name=f"hk{m}{j}",
                                          tag="hk")
                            nc.vector.tensor_add(
                                hk[:], pk[:],
                                bias_bc[:, (6 + j) * DS:(7 + j) * DS])
                            _tr(nc, pstr3, f"hk_{m}_{j}", hk[:], ident[:],
                                [(hkT[:, j, m * 128:(m + 1) * 128], "v")])
                    for j, nm in [(0, "fkn"), (1, "rkn")]:
                        w_kn[nm] = []
                        for m in range(MT):
                            wt = st3.tile([128, N], F32, name=f"wkn{j}_{m}")
                            _routing(nc, rtp, psmm3, f"rk{j}_{m}",
                                     hkT[:, j, m * 128:(m + 1) * 128],
                                     et_sb[:, (4 + j) * N:(5 + j) * N], wt[:])
                            w_kn[nm].append(wt)
                    if dbg:
                        for nm, key in [("w_fknow", "fkn"), ("w_rknow", "rkn")]:
                            td = dbg_tensor(nm, [TL, N])
                            for m in range(MT):
                                nc.sync.dma_start(td[m * 128:(m + 1) * 128, :],
                                                  w_kn[key][m][:])
                    for m in range(MT):
                        _tr(nc, pstr3, f"wt_kn_{m}", w_kn["rkn"][m][:],
                            ident[:], [(wtt_kn[:, m * 128:(m + 1) * 128], "v")])
                    nc.gpsimd.dma_start(wt_dram["rkn"][0], wtt_kn[:])

                    _feature(nc, fp2, psf3, fkn_d, nx2T_r,
                             [(w_kn["fkn"], h_kn)],
                             [nc.sync, nc.scalar, nc.gpsimd])
                    if dbg:
                        td = dbg_tensor("h_know", [TL, R])
                        for m in range(MT):
                            nc.sync.dma_start(td[m * 128:(m + 1) * 128, :],
                                              h_kn[m][:])
                    for m in range(MT):
                        for rb in range(4):
                            _tr(nc, pstr3, f"hkn_{m}_{rb}",
                                h_kn[m][:, rb * 128:(rb + 1) * 128], ident[:],
                                [(hT_kn[:, rb, m * 128:(m + 1) * 128], "v")])

                pools3 = {"rchunk": rp2, "wb": wbp2, "gt": gtp2}
                with tc.tile_pool(name="ps_y3", bufs=1, space="PSUM") as psy3:
                    _restore(nc, pools3, rkn_d,
                             [(hT_kn[:], wt_dram["rkn"][0], yT_kn[:])],
                             psy3, [nc.sync, nc.scalar])

                out_sb = st3.tile([128, MT, D], F32)
                with tc.tile_pool(name="ps_fin", bufs=2, space="PSUM") as psfin:
                    for dt in range(KT):
                        for m in range(MT):
                            p = psfin.tile([128, 128], F32,
                                           name=f"fin_{dt}_{m}", tag="fin")
                            nc.tensor.transpose(
                                p[:], yT_kn[:, dt, m * 128:(m + 1) * 128],
                                ident[:])
                            nc.vector.tensor_add(
                                out_sb[:, m, dt * 128:(dt + 1) * 128], p[:],
                                x2[:, m, dt * 128:(dt + 1) * 128])
                for m in range(MT):
                    nc.sync.dma_start(y_d[m * 128:(m + 1) * 128, :],
                                      out_sb[:, m, :])
            late_cm.__exit__(None, None, None)

    nc.compile()
    return nc, dbg_t


def prep_inputs(inputs):
    f32 = np.float32
    bf16 = mybir.dt.np(BF16)
    x = np.ascontiguousarray(np.asarray(inputs["x"], f32).reshape(T, D))
    ne = np.asarray(inputs["neuron_emb"], f32)
    emb = ne / (np.linalg.norm(ne, axis=-1, keepdims=True) + 1e-8)

    def f_layout(f):
        f = np.asarray(f, f32)
        return np.ascontiguousarray(
            f.reshape(N, KT, 128, R).transpose(2, 0, 1, 3)
            .reshape(128, N, KT * R).astype(bf16))

    def r_layout(r):
        r = np.asarray(r, f32).reshape(N * R, D)
        return np.ascontiguousarray(
            r.reshape(NRT, 128, D).transpose(1, 0, 2).astype(bf16))

    def w_layout(w):
        w = np.asarray(w, f32)
        return np.ascontiguousarray(
            w.reshape(KT, 128, w.shape[-1]).transpose(1, 0, 2))

    shared = {
        "wall": w_layout(inputs["W_all"]),
        "wo": w_layout(inputs["W_o"]),
        "wfk": w_layout(inputs["W_fk"]),
        "wrk": w_layout(inputs["W_rk"]),
        "et": np.ascontiguousarray(emb.T),
        "fqk": f_layout(inputs["f_qk"]),
        "fv": f_layout(inputs["f_v"]),
        "fkn": f_layout(inputs["f_know"]),
        "rqk": r_layout(inputs["r_qk"]),
        "rv": r_layout(inputs["r_v"]),
        "rkn": r_layout(inputs["r_know"]),
        "lnrows": np.ascontiguousarray(
            np.stack([np.asarray(inputs[k], f32)
                      for k in ("ln1_s", "ln1_b", "ln2_s", "ln2_b")])),
        "biasrow": np.ascontiguousarray(
            np.concatenate([np.asarray(inputs["b_all"], f32),
                            np.asarray(inputs["b_fk"], f32),
                            np.asarray(inputs["b_rk"], f32)])[None, :]),
    }
    per_core = []
    k_idx = np.arange(S)[:, None]
    for c in range(NCORES):
        ci = c % (S // TL)
        q_idx = ci * TL + np.arange(TL)[None, :]
        maskT = np.where(k_idx <= q_idx, 0.0, NEG).astype(f32)
        per_core.append({
            "x": np.ascontiguousarray(x[c * TL:(c + 1) * TL]),
            "maskT": np.ascontiguousarray(maskT),
            **shared,
        })
    return per_core


def kernel(**inputs):
    global _PROG
    if _PROG is None:
        _PROG = build(dbg=False)
    nc, _ = _PROG
    per_core = prep_inputs(inputs)
    res = run_bass_kernel_spmd(nc, per_core, core_ids=list(range(NCORES)))
    y = np.concatenate([res.results[c]["y"] for c in range(NCORES)], axis=0)
    return y.reshape(B, S, D).astype(np.float32)



# revision 59
# speedup vs baseline: 1.8964x; 1.0017x over previous
"""Trainium2 Bass kernel for the moe_routing problem (nn_DAWN_69904887709893).

Token-parallel across 8 NeuronCores (256 tokens/core), neuron pools replicated.
Heavy einsums (feature/restore/attention) run in fp32r on the PE; every matmul
feeding router logits (W_all, logit projections, W_fk/W_rk, W_o) runs in plain
fp32 so top-k selections match the fp32 reference. K^T and token-major V are
exchanged with a 4-rank AllGather per sequence group; causality comes from a
host-provided additive mask so the SPMD program is identical on every core.

PSUM accumulators that pack two regions per bank are pre-zeroed with memset and
use start=False matmuls throughout: correct regardless of has_written state and
immune to instruction reordering (a start=True clears the whole bank's bits).
"""
import sys

sys.path.insert(0, "/opt/trn_rl_repo")
import numpy as np
import concourse.bass as bass
import concourse.bacc as bacc
import concourse.mybir as mybir
import concourse.tile as tile
from concourse.bass_utils import run_bass_kernel_spmd
from concourse.masks import make_identity

F32 = mybir.dt.float32
F32R = mybir.dt.float32r
BF16 = mybir.dt.bfloat16
AX = mybir.AxisListType.X
OP = mybir.AluOpType
ACT = mybir.ActivationFunctionType

NCORES = 8
B, S, D, R, N, DS, TOPK, H = 2, 1024, 1024, 512, 32, 64, 4, 16
T = B * S
TL = T // NCORES          # tokens per core (256)
MT = TL // 128            # token tiles per core (2)
KT = D // 128             # contraction tiles over D (8)
DH = D // H               # head dim (64)
NRT = (N * R) // 128      # contraction tiles over N*R (128)
SEQ_BLOCKS = S // 128     # k blocks per sequence (8)
NEG = -1.0e30

# emb segment used by each of the 6 attention routings (fq, fk, fv, rq, rk, rv)
ATTN_SEG = [0, 0, 1, 2, 2, 3]

_PROG = None


def _routing(nc, rt, psmm, name, lhsT_ap, e_ap, wout):
    """softmax over 32 logits + top-4 sparsify + renormalize -> wout [128,32] f32.
    lhsT_ap: [64, 128] fp32 (h-segment transposed), e_ap: [64, 32] fp32."""
    lg = psmm.tile([128, N], F32, name=f"lg_{name}", tag="mm")
    nc.tensor.matmul(lg[:], lhsT_ap, e_ap, start=True, stop=True)
    mx = rt.tile([128, 1], F32, name=f"mx_{name}", tag="mx")
    nc.vector.tensor_reduce(mx[:], lg[:], AX, OP.max)
    nmx = rt.tile([128, 1], F32, name=f"nmx_{name}", tag="nmx")
    nc.scalar.mul(nmx[:], mx[:], -1.0)
    ex = rt.tile([128, N], F32, name=f"ex_{name}", tag="ex")
    ssum = rt.tile([128, 1], F32, name=f"ssum_{name}", tag="ssum")
    nc.scalar.activation(ex[:], lg[:], ACT.Exp, bias=nmx[:], scale=1.0,
                         accum_out=ssum[:])
    rs = rt.tile([128, 1], F32, name=f"rs_{name}", tag="rs")
    nc.vector.reciprocal(rs[:], ssum[:])
    sm = rt.tile([128, N], F32, name=f"sm_{name}", tag="sm")
    nc.vector.tensor_scalar_mul(sm[:], ex[:], rs[:])
    top8 = rt.tile([128, 8], F32, name=f"top8_{name}", tag="top8")
    nc.vector.max(top8[:], sm[:])
    ge = rt.tile([128, N], F32, name=f"ge_{name}", tag="ge")
    nc.vector.tensor_scalar(ge[:], sm[:], top8[:, 3:4], None, OP.is_ge)
    sp = rt.tile([128, N], F32, name=f"sp_{name}", tag="sp")
    nc.vector.tensor_mul(sp[:], sm[:], ge[:])
    s2 = rt.tile([128, 1], F32, name=f"s2_{name}", tag="s2")
    nc.vector.tensor_reduce(s2[:], sp[:], AX, OP.add)
    s2e = rt.tile([128, 1], F32, name=f"s2e_{name}", tag="s2e")
    nc.vector.tensor_scalar_add(s2e[:], s2[:], 1e-8)
    rs2 = rt.tile([128, 1], F32, name=f"rs2_{name}", tag="rs2")
    nc.vector.reciprocal(rs2[:], s2e[:])
    nc.vector.tensor_scalar_mul(wout, sp[:], rs2[:])


def _layernorm(nc, lnp, name, x_ap, s_bc, b_bc, out_ap):
    """LN over the free dim (D). x_ap/out_ap [128, D] f32; s_bc/b_bc [128, D]."""
    mu = lnp.tile([128, 1], F32, name=f"mu_{name}", tag="mu")
    nc.vector.tensor_reduce(mu[:], x_ap, AX, OP.add)
    nc.scalar.mul(mu[:], mu[:], 1.0 / D)
    xc = lnp.tile([128, D], F32, name=f"xc_{name}", tag="xc")
    nc.vector.tensor_scalar_sub(xc[:], x_ap, mu[:])
    sq = lnp.tile([128, D], F32, name=f"sq_{name}", tag="sq")
    vs = lnp.tile([128, 1], F32, name=f"vs_{name}", tag="vs")
    nc.scalar.activation(sq[:], xc[:], ACT.Square, accum_out=vs[:])
    nc.scalar.activation(vs[:], vs[:], ACT.Copy, scale=1.0 / D, bias=1e-6)
    rv = lnp.tile([128, 1], F32, name=f"rv_{name}", tag="rv")
    nc.vector.reciprocal(rv[:], vs[:])
    rstd = lnp.tile([128, 1], F32, name=f"rstd_{name}", tag="rstd")
    nc.scalar.activation(rstd[:], rv[:], ACT.Sqrt)
    nc.vector.scalar_tensor_tensor(out_ap, xc[:], rstd[:], s_bc, OP.mult, OP.mult)
    nc.vector.tensor_add(out_ap, out_ap, b_bc)


def _tr(nc, pstr, name, src_ap, ident, outs):
    """PE-transpose a [128, <=128] block; copy the psum into each (ap, engine)."""
    p = pstr.tile([src_ap.shape[-1], 128], src_ap.dtype, name=f"tr_{name}",
                  tag="tr")
    nc.tensor.transpose(p[:], src_ap, ident)
    for ap, eng in outs:
        if eng == "v":
            nc.vector.tensor_copy(ap, p[:, :ap.shape[-1]])
        else:
            nc.scalar.copy(ap, p[:, :ap.shape[-1]])


def _feature(nc, fp, psf, fdram, nxT_r, routes, engs):
    """h[m] accumulators += w[:,n] * (nx @ f_n) for all 32 neurons.
    routes: list of (w_tiles_per_m, hacc_per_m). F is streamed in half-neuron
    chunks (4 k-tiles each) to halve SBUF residency."""
    HKT = KT // 2
    for n in range(N):
        pfs = [psf.tile([128, R], F32, name=f"pf{m}", tag=f"pf{m}")
               for m in range(MT)]
        for half in range(2):
            fc = fp.tile([128, HKT * R], fdram.dtype, name="fc", tag="fc")
            engs[(2 * n + half) % len(engs)].dma_start(
                fc[:], fdram[:, n, half * HKT * R:(half + 1) * HKT * R])
            for m in range(MT):
                for kk in range(HKT):
                    k = half * HKT + kk
                    nc.tensor.matmul(pfs[m][:],
                                     nxT_r[:, k, m * 128:(m + 1) * 128],
                                     fc[:, kk * R:(kk + 1) * R],
                                     start=(k == 0), stop=(k == KT - 1))
        for m in range(MT):
            for wt, hacc in routes:
                w_ap = wt[m][:, n:n + 1]
                if n == 0:
                    nc.vector.tensor_scalar(hacc[m][:], pfs[m][:], w_ap, None,
                                            OP.mult)
                else:
                    nc.vector.scalar_tensor_tensor(hacc[m][:], pfs[m][:], w_ap,
                                                   hacc[m][:], OP.mult, OP.add)


def _restore(nc, pools, rdram, routes, psy, engs):
    """Transposed-output restores sharing one streamed r matrix.
    routes: list of (hT_tile, w_sb [N, TL] sbuf tile, yT_out_ap).
    Packs len(routes) [128, TL] accumulators per PSUM bank; all matmuls use
    start=False on memset-zeroed banks."""
    nr = len(routes)
    assert nr in (1, 2)
    per_bank = 2
    n_banks = (KT * nr + per_bank - 1) // per_bank
    pys = [psy.tile([128, per_bank, TL], F32, name=f"ry{i}", tag=f"ry{i}")
           for i in range(n_banks)]
    for py in pys:
        nc.vector.memset(py[:], 0.0)

    def acc_ap(ri, dt):
        flat = dt * nr + ri
        return pys[flat // per_bank][:, flat % per_bank, :]

    rp, wbp, gtp = pools["rchunk"], pools["wb"], pools["gt"]
    for kt in range(NRT):
        n, rb = kt // 4, kt % 4
        wbs = []
        for ri, (hT, wtd, _) in enumerate(routes):
            if rb == 0:
                wb = wbp.tile([128, TL], F32, name=f"wb{ri}", tag=f"wb{ri}")
                nc.gpsimd.dma_start(wb[:],
                                    wtd[n:n + 1, :].broadcast_to([128, TL]))
                pools[f"_wb{ri}"] = wb
            wbs.append(pools[f"_wb{ri}"])
        if kt % 2 == 0:
            rc = rp.tile([128, 2, D], rdram.dtype, name="rc", tag="rc")
            engs[(kt // 2) % len(engs)].dma_start(rc[:], rdram[:, kt:kt + 2, :])
            pools["_rc"] = rc
        rc = pools["_rc"]
        for ri, (hT, _, _) in enumerate(routes):
            gt = gtp.tile([128, TL], rdram.dtype, name=f"gt{ri}", tag=f"gt{ri}")
            nc.vector.tensor_mul(gt[:], hT[:, rb, :], wbs[ri][:])
            for dt in range(KT):
                nc.tensor.matmul(acc_ap(ri, dt),
                                 rc[:, kt % 2, dt * 128:(dt + 1) * 128],
                                 gt[:], start=False, stop=(kt == NRT - 1))
    for ri, (_, _, yT_out) in enumerate(routes):
        for dt in range(KT):
            eng = nc.scalar if (dt + ri) % 2 == 0 else nc.vector
            if eng is nc.scalar:
                nc.scalar.copy(yT_out[:, dt, :], acc_ap(ri, dt))
            else:
                nc.vector.tensor_copy(yT_out[:, dt, :], acc_ap(ri, dt))


def build(dbg=False):
    nc = bacc.Bacc("TRN2", target_bir_lowering=False, debug=False,
                   num_devices=NCORES)

    x_d = nc.dram_tensor("x", [TL, D], F32, kind="ExternalInput")
    maskT_d = nc.dram_tensor("maskT", [S, TL], F32, kind="ExternalInput")
    wall_d = nc.dram_tensor("wall", [128, KT, 6 * DS], F32, kind="ExternalInput")
    wo_d = nc.dram_tensor("wo", [128, KT, D], F32R, kind="ExternalInput")
    wfk_d = nc.dram_tensor("wfk", [128, KT, DS], F32, kind="ExternalInput")
    wrk_d = nc.dram_tensor("wrk", [128, KT, DS], F32, kind="ExternalInput")
    et_d = nc.dram_tensor("et", [DS, 6 * N], F32, kind="ExternalInput")
    fqk_d = nc.dram_tensor("fqk", [128, N, KT * R], BF16, kind="ExternalInput")
    fv_d = nc.dram_tensor("fv", [128, N, KT * R], BF16, kind="ExternalInput")
    fkn_d = nc.dram_tensor("fkn", [128, N, KT * R], BF16, kind="ExternalInput")
    rqk_d = nc.dram_tensor("rqk", [128, NRT, D], BF16, kind="ExternalInput")
    rv_d = nc.dram_tensor("rv", [128, NRT, D], BF16, kind="ExternalInput")
    rkn_d = nc.dram_tensor("rkn", [128, NRT, D], BF16, kind="ExternalInput")
    ln_d = nc.dram_tensor("lnrows", [4, D], F32, kind="ExternalInput")
    bias_d = nc.dram_tensor("biasrow", [1, 8 * DS], F32, kind="ExternalInput")
    y_d = nc.dram_tensor("y", [TL, D], F32, kind="ExternalOutput")

    dbg_t = {}

    def dbg_tensor(name, shape):
        dbg_t[name] = nc.dram_tensor("dbg_" + name, shape, F32,
                                     kind="ExternalOutput")
        return dbg_t[name]

    with tile.TileContext(nc) as tc:
        with (
            tc.tile_pool(name="perm", bufs=1) as perm,
            tc.tile_pool(name="dramp", bufs=1, space="DRAM") as dramp,
            tc.tile_pool(name="lnp", bufs=1) as lnp,
            tc.tile_pool(name="rtp", bufs=2) as rtp,
        ):
            # collective bounce buffers (K^T gathered first, then token-major V)
            cck_in = dramp.tile([128, KT * TL], BF16, name="cck_in")
            cck_out = dramp.tile([4 * 128, KT * TL], BF16, name="cck_out")
            ccv_in = dramp.tile([128, MT * D], BF16, name="ccv_in")
            ccv_out = dramp.tile([4 * 128, MT * D], BF16, name="ccv_out")
            # restore-route w rows, bounced through DRAM into one partition
            wt_dram = {k: dramp.tile([1, N, TL], F32, name=f"wtd_{k}")
                       for k in ("rq", "rk", "rv", "rkn")}

            ident = perm.tile([128, 128], F32)
            make_identity(nc, ident[:])
            ones_f = perm.tile([128, 1], F32)
            nc.gpsimd.memset(ones_f[:], 1.0)
            ones_b = perm.tile([128, 1], BF16)
            nc.vector.tensor_copy(ones_b[:], ones_f[:])
            bias_bc = perm.tile([128, 8 * DS], F32)
            nc.sync.dma_start(bias_bc[:], bias_d[0:1, :].broadcast_to([128, 8 * DS]))
            et_sb = perm.tile([DS, 6 * N], F32)
            nc.sync.dma_start(et_sb[:], et_d[:])
            # copy at partition base 64 for routings whose h-segment sits in
            # the upper half of a transposed tile (matmul requires equal bases)
            et_hi = perm.tile([128, 6 * N], F32)
            nc.sync.dma_start(et_hi[DS:2 * DS, :], et_d[:])
            x_sb = perm.tile([128, MT, D], F32)
            for m in range(MT):
                nc.sync.dma_start(x_sb[:, m, :], x_d[m * 128:(m + 1) * 128, :])
            maskT_sb = perm.tile([128, SEQ_BLOCKS, TL], F32)
            yT_q = perm.tile([128, KT, TL], BF16)

            # ============ stage 1: LN1 + routing + features + restores ========
            with (
                tc.tile_pool(name="st1", bufs=1) as st1,
                tc.tile_pool(name="fchunk", bufs=4) as fp,
                tc.tile_pool(name="rchunk", bufs=4) as rp,
                tc.tile_pool(name="wbp", bufs=2) as wbp,
                tc.tile_pool(name="gtp", bufs=3) as gtp,
            ):
                nxT_r = st1.tile([128, KT, TL], BF16)
                h_q = [st1.tile([128, R], F32, name=f"h_q{m}") for m in range(MT)]
                h_k = [st1.tile([128, R], F32, name=f"h_k{m}") for m in range(MT)]
                h_v = [st1.tile([128, R], F32, name=f"h_v{m}") for m in range(MT)]
                hT = {k: st1.tile([128, 4, TL], F32, name=f"hT_{k}")
                      for k in ("q", "k", "v")}
                w_feat = {p: [st1.tile([128, N], F32, name=f"w{p}_{m}")
                              for m in range(MT)] for p in range(3)}
                wtt_sb = {k: st1.tile([N, TL], F32, name=f"wtt_{k}")
                          for k in ("rq", "rk", "rv")}
                yT_k = st1.tile([128, KT, TL], BF16)
                v_tok = st1.tile([128, MT, D], BF16)

                with (
                    tc.tile_pool(name="st1a", bufs=1) as st1a,
                    tc.tile_pool(name="ps_tr", bufs=2, space="PSUM") as pstr,
                    tc.tile_pool(name="ps_mm", bufs=2, space="PSUM") as psmm,
                    tc.tile_pool(name="ps_feat", bufs=2, space="PSUM") as psf,
                    tc.tile_pool(name="wallp", bufs=2) as wallp,
                ):
                    nxT = st1a.tile([128, KT, TL], F32)
                    nx = st1a.tile([128, MT, D], F32)
                    ln1_bc = st1a.tile([128, 2, D], F32)
                    for i in range(2):
                        nc.gpsimd.dma_start(ln1_bc[:, i, :],
                                            ln_d[i:i + 1, :]
                                            .broadcast_to([128, D]))
                    for m in range(MT):
                        _layernorm(nc, lnp, f"ln1_{m}", x_sb[:, m, :],
                                   ln1_bc[:, 0, :], ln1_bc[:, 1, :], nx[:, m, :])
                    if dbg:
                        td = dbg_tensor("nx", [TL, D])
                        for m in range(MT):
                            nc.sync.dma_start(td[m * 128:(m + 1) * 128, :],
                                              nx[:, m, :])
                    for m in range(MT):
                        for k in range(KT):
                            _tr(nc, pstr, f"nx_{m}_{k}",
                                nx[:, m, k * 128:(k + 1) * 128], ident[:],
                                [(nxT[:, k, m * 128:(m + 1) * 128], "v"),
                                 (nxT_r[:, k, m * 128:(m + 1) * 128], "s")])

                    hall = st1a.tile([128, MT, 6 * DS], F32)
                    phs = [psmm.tile([128, 6 * DS], F32, name=f"ph{m}",
                                     tag="mm") for m in range(MT)]
                    for k in range(KT):
                        wt_k = wallp.tile([128, 6 * DS], F32, name="wal",
                                          tag="wal")
                        [nc.sync, nc.scalar][k % 2].dma_start(
                            wt_k[:], wall_d[:, k, :])
                        for m in range(MT):
                            nc.tensor.matmul(phs[m][:],
                                             nxT[:, k, m * 128:(m + 1) * 128],
                                             wt_k[:],
                                             start=(k == 0), stop=(k == KT - 1))
                    for m in range(MT):
                        nc.vector.tensor_add(hall[:, m, :], phs[m][:],
                                             bias_bc[:, :6 * DS])
                    # prefetch the attention mask well before stage 2
                    for kb in range(SEQ_BLOCKS):
                        nc.gpsimd.dma_start(maskT_sb[:, kb, :],
                                            maskT_d[kb * 128:(kb + 1) * 128, :])
                    hallT = st1a.tile([128, 3, TL], F32)
                    for m in range(MT):
                        for i in range(3):
                            _tr(nc, pstr, f"ha_{m}_{i}",
                                hall[:, m, i * 128:(i + 1) * 128], ident[:],
                                [(hallT[:, i, m * 128:(m + 1) * 128], "v")])
                    w_rest = {}
                    for p in range(6):
                        seg = ATTN_SEG[p]
                        tiles = w_feat[p] if p < 3 else \
                            [st1a.tile([128, N], F32, name=f"w{p}_{m}")
                             for m in range(MT)]
                        if p >= 3:
                            w_rest[p] = tiles
                        for m in range(MT):
                            base, ti = (p % 2) * DS, p // 2
                            e_src = et_sb if base == 0 else et_hi
                            e_ap = e_src[base:base + DS,
                                         seg * N:(seg + 1) * N]
                            _routing(nc, rtp, psmm, f"r{p}_{m}",
                                     hallT[base:base + DS, ti,
                                           m * 128:(m + 1) * 128],
                                     e_ap, tiles[m][:])
                    if dbg:
                        nm6 = ["w_fq", "w_fk", "w_fv", "w_rq", "w_rk", "w_rv"]
                        for p in range(6):
                            td = dbg_tensor(nm6[p], [TL, N])
                            tiles = w_feat[p] if p < 3 else w_rest[p]
                            for m in range(MT):
                                nc.sync.dma_start(td[m * 128:(m + 1) * 128, :],
                                                  tiles[m][:])
                    for p, key in [(3, "rq"), (4, "rk"), (5, "rv")]:
                        for m in range(MT):
                            _tr(nc, pstr, f"wt_{p}_{m}", w_rest[p][m][:],
                                ident[:],
                                [(wtt_sb[key][:, m * 128:(m + 1) * 128], "v")])
                        nc.gpsimd.dma_start(wt_dram[key][0], wtt_sb[key][:])

                    # features (qk shared for Q and K; v)
                    _feature(nc, fp, psf, fqk_d, nxT_r,
                             [(w_feat[0], h_q), (w_feat[1], h_k)],
                             [nc.sync, nc.scalar])
                    _feature(nc, fp, psf, fv_d, nxT_r,
                             [(w_feat[2], h_v)], [nc.sync, nc.scalar])
                    if dbg:
                        for nm, hh in [("h_q", h_q), ("h_k", h_k), ("h_v", h_v)]:
                            td = dbg_tensor(nm, [TL, R])
                            for m in range(MT):
                                nc.sync.dma_start(td[m * 128:(m + 1) * 128, :],
                                                  hh[m][:])
                    for nm, hh in [("q", h_q), ("k", h_k), ("v", h_v)]:
                        for m in range(MT):
                            for rb in range(4):
                                _tr(nc, pstr, f"h{nm}_{m}_{rb}",
                                    hh[m][:, rb * 128:(rb + 1) * 128], ident[:],
                                    [(hT[nm][:, rb, m * 128:(m + 1) * 128], "v")])

                # restores: Q+K fused (stream r_qk once), then V
                pools = {"rchunk": rp, "wb": wbp, "gt": gtp}
                with tc.tile_pool(name="ps_y", bufs=1, space="PSUM") as psy:
                    _restore(nc, pools, rqk_d,
                             [(hT["q"][:], wt_dram["rq"][0], yT_q[:]),
                              (hT["k"][:], wt_dram["rk"][0], yT_k[:])],
                             psy, [nc.sync, nc.scalar])
                # K^T gather starts while the V restore computes
                nc.sync.dma_start(cck_in[:],
                                  yT_k[:].rearrange("p k t -> p (k t)"))
                nc.gpsimd.collective_compute(
                    "AllGather", OP.bypass,
                    ins=[cck_in[:]],
                    outs=[cck_out[:]],
                    replica_groups=[[0, 1, 2, 3], [4, 5, 6, 7]],
                )
                # V restore in token-major orientation: psum accumulators are
                # [tok, d-half]; v_tok comes straight out of PSUM (no
                # transposes on the V collective's critical path).
                with tc.tile_pool(name="ps_y2", bufs=1, space="PSUM") as psy:
                    pvs = [[psy.tile([128, 512], F32, name=f"pv{m}{db}",
                                     tag=f"pv{m}{db}") for db in range(2)]
                           for m in range(MT)]
                    for m in range(MT):
                        for db in range(2):
                            nc.vector.memset(pvs[m][db][:], 0.0)
                    for kt in range(NRT):
                        n, rb = kt // 4, kt % 4
                        if rb == 0:
                            wbv = wbp.tile([128, TL], F32, name="wbv",
                                           tag="wb0")
                            nc.gpsimd.dma_start(
                                wbv[:], wt_dram["rv"][0][n:n + 1, :]
                                .broadcast_to([128, TL]))
                            pools["_wbv"] = wbv
                        if kt % 2 == 0:
                            rcv = rp.tile([128, 2, D], rv_d.dtype, name="rcv",
                                          tag="rc")
                            [nc.sync, nc.scalar][(kt // 2) % 2].dma_start(
                                rcv[:], rv_d[:, kt:kt + 2, :])
                            pools["_rcv"] = rcv
                        rcv = pools["_rcv"]
                        gtv = gtp.tile([128, TL], rv_d.dtype, name="gtv",
                                       tag="gt0")
                        nc.vector.tensor_mul(gtv[:], hT["v"][:, rb, :],
                                             pools["_wbv"][:])
                        for m in range(MT):
                            for db in range(2):
                                nc.tensor.matmul(
                                    pvs[m][db][:],
                                    gtv[:, m * 128:(m + 1) * 128],
                                    rcv[:, kt % 2, db * 512:(db + 1) * 512],
                                    start=False, stop=(kt == NRT - 1))
                    for m in range(MT):
                        for db in range(2):
                            dst = v_tok[:, m, db * 512:(db + 1) * 512]
                            if (m + db) % 2 == 0:
                                nc.scalar.copy(dst, pvs[m][db][:])
                            else:
                                nc.vector.tensor_copy(dst, pvs[m][db][:])
                for m in range(MT):
                    nc.sync.dma_start(ccv_in[:, m * D:(m + 1) * D],
                                      v_tok[:, m, :])
            nc.gpsimd.collective_compute(
                "AllGather", OP.bypass,
                ins=[ccv_in[:]],
                outs=[ccv_out[:]],
                replica_groups=[[0, 1, 2, 3], [4, 5, 6, 7]],
            )

            # ============ stage 2: attention + W_o ============
            late_cm = tc.tile_pool(name="late", bufs=1)
            late = late_cm.__enter__()
            x2 = late.tile([128, MT, D], F32)
            ot_sb = late.tile([128, KT, TL], F32)
            with (
                tc.tile_pool(name="st2", bufs=1) as st2,
                tc.tile_pool(name="attp", bufs=3) as att,
                tc.tile_pool(name="ps_att", bufs=2, space="PSUM") as psa,
                tc.tile_pool(name="ps_ot", bufs=4, space="PSUM") as psot,
            ):
                # phase A: scores + exp for all heads (K only) — the V
                # AllGather and v_all loads hide under this pass.
                expt_all = st2.tile([128, 2 * KT, SEQ_BLOCKS, TL], BF16)
                with tc.tile_pool(name="ktp", bufs=1) as ktp:
                    kt_all = ktp.tile([128, 4, KT * TL], BF16)
                    ld_engs = [nc.sync, nc.scalar, nc.gpsimd]
                    for ch in range(4):
                        ld_engs[ch % 3].dma_start(
                            kt_all[:, ch, :],
                            cck_out[ch * 128:(ch + 1) * 128, :])
                    for hp in range(KT):
                        for hh in range(2):
                            h_idx = hp * 2 + hh
                            qt_ap = yT_q[hh * DH:(hh + 1) * DH, hp, :]
                            for kb in range(SEQ_BLOCKS):
                                ch, m2 = kb // 2, kb % 2
                                ktap = kt_all[hh * DH:(hh + 1) * DH, ch,
                                              hp * TL + m2 * 128:
                                              hp * TL + (m2 + 1) * 128]
                                pscore = psa.tile([128, TL], F32,
                                                  name="pscore", tag="pscore")
                                nc.tensor.matmul(pscore[:], ktap, qt_ap,
                                                 start=True, stop=True)
                                msc = att.tile([128, TL], F32, name="msc",
                                               tag="msc")
                                nc.vector.tensor_add(msc[:], pscore[:],
                                                     maskT_sb[:, kb, :])
                                nc.scalar.activation(
                                    expt_all[:, h_idx, kb, :], msc[:],
                                    ACT.Exp, scale=0.125)
                # phase B: softmax denominators + AV
                v_all = st2.tile([128, 4, MT * D], BF16)
                ld_engs = [nc.sync, nc.scalar, nc.gpsimd]
                for ch in range(4):
                    ld_engs[ch % 3].dma_start(
                        v_all[:, ch, :], ccv_out[ch * 128:(ch + 1) * 128, :])
                for hp in range(KT):
                    for hh in range(2):
                        h_idx = hp * 2 + hh
                        pot = psot.tile([DH, TL], F32, name="pot", tag="pot")
                        nc.vector.memset(pot[:], 0.0)
                        pss = psa.tile([1, TL], F32, name="pss", tag="pss")
                        nc.vector.memset(pss[:], 0.0)
                        for kb in range(SEQ_BLOCKS):
                            ch, m2 = kb // 2, kb % 2
                            vap = v_all[:, ch,
                                        m2 * D + h_idx * DH:
                                        m2 * D + (h_idx + 1) * DH]
                            nc.tensor.matmul(pss[:], ones_b[:],
                                             expt_all[:, h_idx, kb, :],
                                             start=False,
                                             stop=(kb == SEQ_BLOCKS - 1))
                            nc.tensor.matmul(pot[:], vap,
                                             expt_all[:, h_idx, kb, :],
                                             start=False,
                                             stop=(kb == SEQ_BLOCKS - 1))
                        rsr = att.tile([1, TL], F32, name="rsr", tag="rsr")
                        nc.vector.reciprocal(rsr[:], pss[:])
                        rbc = att.tile([DH, TL], F32, name="rbc", tag="rbc")
                        nc.gpsimd.partition_broadcast(rbc[:], rsr[:],
                                                      channels=DH)
                        otn = att.tile([DH, TL], F32, name="otn", tag="otn")
                        nc.vector.tensor_mul(otn[:], pot[:], rbc[:])
                        # SBUF->SBUF DMA can shift partitions (DVE cannot)
                        nc.sync.dma_start(ot_sb[hh * DH:(hh + 1) * DH, hp, :],
                                          otn[:])

            with (
                tc.tile_pool(name="wop", bufs=3) as wop,
                tc.tile_pool(name="ps_mm2", bufs=2, space="PSUM") as psmm2,
            ):
                ot_r = ot_sb[:].bitcast(F32R)
                for blk in range(2):
                    wo_t = []
                    for k in range(KT):
                        wt_k = wop.tile([128, 512], F32R, name=f"wo{k}",
                                        tag="wo")
                        [nc.sync, nc.scalar][k % 2].dma_start(
                            wt_k[:], wo_d[:, k, blk * 512:(blk + 1) * 512])
                        wo_t.append(wt_k)
                    for m in range(MT):
                        px = psmm2.tile([128, 512], F32, name="px", tag="mm")
                        for k in range(KT):
                            nc.tensor.matmul(px[:],
                                             ot_r[:, k, m * 128:(m + 1) * 128],
                                             wo_t[k][:],
                                             start=(k == 0), stop=(k == KT - 1))
                        nc.vector.tensor_add(
                            x2[:, m, blk * 512:(blk + 1) * 512], px[:],
                            x_sb[:, m, blk * 512:(blk + 1) * 512])
            if dbg:
                td = dbg_tensor("x2", [TL, D])
                for m in range(MT):
                    nc.sync.dma_start(td[m * 128:(m + 1) * 128, :], x2[:, m, :])

            # ============ stage 3: knowledge circuit ============
            with (
                tc.tile_pool(name="st3", bufs=1) as st3,
                tc.tile_pool(name="fchunk2", bufs=3) as fp2,
                tc.tile_pool(name="rchunk2", bufs=3) as rp2,
                tc.tile_pool(name="wbp2", bufs=2) as wbp2,
                tc.tile_pool(name="gtp2", bufs=3) as gtp2,
            ):
                nx2T_r = st3.tile([128, KT, TL], BF16)
                h_kn = [st3.tile([128, R], F32, name=f"h_kn{m}")
                        for m in range(MT)]
                hT_kn = st3.tile([128, 4, TL], F32)
                wtt_kn = st3.tile([N, TL], F32, name="wtt_kn")
                w_kn = {}
                yT_kn = st3.tile([128, KT, TL], F32)
                with (
                    tc.tile_pool(name="st3a", bufs=1) as st3a,
                    tc.tile_pool(name="ps_tr3", bufs=2, space="PSUM") as pstr3,
                    tc.tile_pool(name="ps_mm3", bufs=2, space="PSUM") as psmm3,
                    tc.tile_pool(name="ps_feat3", bufs=2, space="PSUM") as psf3,
                ):
                    nx2 = st3a.tile([128, MT, D], F32)
                    ln2_bc = st3a.tile([128, 2, D], F32)
                    for i in range(2):
                        nc.sync.dma_start(
                            ln2_bc[:, i, :],
                            ln_d[i + 2:i + 3, :].broadcast_to([128, D]))
                    for m in range(MT):
                        _layernorm(nc, lnp, f"ln2_{m}", x2[:, m, :],
                                   ln2_bc[:, 0, :], ln2_bc[:, 1, :], nx2[:, m, :])
                    nx2T = st3a.tile([128, KT, TL], F32)
                    for m in range(MT):
                        for k in range(KT):
                            _tr(nc, pstr3, f"nx2_{m}_{k}",
                                nx2[:, m, k * 128:(k + 1) * 128], ident[:],
                                [(nx2T[:, k, m * 128:(m + 1) * 128], "v"),
                                 (nx2T_r[:, k, m * 128:(m + 1) * 128], "s")])
                    wk_sb = st3a.tile([128, KT, 2 * DS], F32)
                    nc.sync.dma_start(wk_sb[:, :, :DS], wfk_d[:])
                    nc.sync.dma_start(wk_sb[:, :, DS:], wrk_d[:])
                    hkT = st3a.tile([DS, 2, TL], F32)
                    for m in range(MT):
                        for j in range(2):
                            pk = psmm3.tile([128, DS], F32, name="pk", tag="mm")
                            for k in range(KT):
                                nc.tensor.matmul(
                                    pk[:], nx2T[:, k, m * 128:(m + 1) * 128],
                                    wk_sb[:, k, j * DS:(j + 1) * DS],
                                    start=(k == 0), stop=(k == KT - 1))
                            hk = rtp.tile([128, DS], F32, name=f"hk{m}{j}",
                                          tag="hk")
                            nc.vector.tensor_add(
                                hk[:], pk[:],
                                bias_bc[:, (6 + j) * DS:(7 + j) * DS])
                            _tr(nc, pstr3, f"hk_{m}_{j}", hk[:], ident[:],
                                [(hkT[:, j, m * 128:(m + 1) * 128], "v")])
                    for j, nm in [(0, "fkn"), (1, "rkn")]:
                        w_kn[nm] = []
                        for m in range(MT):
                            wt = st3.tile([128, N], F32, name=f"wkn{j}_{m}")
                            _routing(nc, rtp, psmm3, f"rk{j}_{m}",
                                     hkT[:, j, m * 128:(m + 1) * 128],
                                     et_sb[:, (4 + j) * N:(5 + j) * N], wt[:])
                            w_kn[nm].append(wt)
                    if dbg:
                        for nm, key in [("w_fknow", "fkn"), ("w_rknow", "rkn")]:
                            td = dbg_tensor(nm, [TL, N])
                            for m in range(MT):
                                nc.sync.dma_start(td[m * 128:(m + 1) * 128, :],
                                                  w_kn[key][m][:])
                    for m in range(MT):
                        _tr(nc, pstr3, f"wt_kn_{m}", w_kn["rkn"][m][:],
                            ident[:], [(wtt_kn[:, m * 128:(m + 1) * 128], "v")])
                    nc.gpsimd.dma_start(wt_dram["rkn"][0], wtt_kn[:])

                    _feature(nc, fp2, psf3, fkn_d, nx2T_r,
                             [(w_kn["fkn"], h_kn)],
                             [nc.sync, nc.scalar])
                    if dbg:
                        td = dbg_tensor("h_know", [TL, R])
                        for m in range(MT):
                            nc.sync.dma_start(td[m * 128:(m + 1) * 128, :],
                                              h_kn[m][:])
                    for m in range(MT):
                        for rb in range(4):
                            _tr(nc, pstr3, f"hkn_{m}_{rb}",
                                h_kn[m][:, rb * 128:(rb + 1) * 128], ident[:],
                                [(hT_kn[:, rb, m * 128:(m + 1) * 128], "v")])

                pools3 = {"rchunk": rp2, "wb": wbp2, "gt": gtp2}
                with tc.tile_pool(name="ps_y3", bufs=1, space="PSUM") as psy3:
                    _restore(nc, pools3, rkn_d,
                             [(hT_kn[:], wt_dram["rkn"][0], yT_kn[:])],
                             psy3, [nc.sync, nc.scalar])

                out_sb = st3.tile([128, MT, D], F32)
                with tc.tile_pool(name="ps_fin", bufs=2, space="PSUM") as psfin:
                    for dt in range(KT):
                        for m in range(MT):
                            p = psfin.tile([128, 128], F32,
                                           name=f"fin_{dt}_{m}", tag="fin")
                            nc.tensor.transpose(
                                p[:], yT_kn[:, dt, m * 128:(m + 1) * 128],
                                ident[:])
                            nc.vector.tensor_add(
                                out_sb[:, m, dt * 128:(dt + 1) * 128], p[:],
                                x2[:, m, dt * 128:(dt + 1) * 128])
                for m in range(MT):
                    nc.sync.dma_start(y_d[m * 128:(m + 1) * 128, :],
                                      out_sb[:, m, :])
            late_cm.__exit__(None, None, None)

    nc.compile()
    return nc, dbg_t


def prep_inputs(inputs):
    f32 = np.float32
    bf16 = mybir.dt.np(BF16)
    x = np.ascontiguousarray(np.asarray(inputs["x"], f32).reshape(T, D))
    ne = np.asarray(inputs["neuron_emb"], f32)
    emb = ne / (np.linalg.norm(ne, axis=-1, keepdims=True) + 1e-8)

    def f_layout(f):
        f = np.asarray(f, f32)
        return np.ascontiguousarray(
            f.reshape(N, KT, 128, R).transpose(2, 0, 1, 3)
            .reshape(128, N, KT * R).astype(bf16))

    def r_layout(r):
        r = np.asarray(r, f32).reshape(N * R, D)
        return np.ascontiguousarray(
            r.reshape(NRT, 128, D).transpose(1, 0, 2).astype(bf16))

    def w_layout(w):
        w = np.asarray(w, f32)
        return np.ascontiguousarray(
            w.reshape(KT, 128, w.shape[-1]).transpose(1, 0, 2))

    shared = {
        "wall": w_layout(inputs["W_all"]),
        "wo": w_layout(inputs["W_o"]),
        "wfk": w_layout(inputs["W_fk"]),
        "wrk": w_layout(inputs["W_rk"]),
        "et": np.ascontiguousarray(emb.T),
        "fqk": f_layout(inputs["f_qk"]),
        "fv": f_layout(inputs["f_v"]),
        "fkn": f_layout(inputs["f_know"]),
        "rqk": r_layout(inputs["r_qk"]),
        "rv": r_layout(inputs["r_v"]),
        "rkn": r_layout(inputs["r_know"]),
        "lnrows": np.ascontiguousarray(
            np.stack([np.asarray(inputs[k], f32)
                      for k in ("ln1_s", "ln1_b", "ln2_s", "ln2_b")])),
        "biasrow": np.ascontiguousarray(
            np.concatenate([np.asarray(inputs["b_all"], f32),
                            np.asarray(inputs["b_fk"], f32),
                            np.asarray(inputs["b_rk"], f32)])[None, :]),
    }
    per_core = []
    k_idx = np.arange(S)[:, None]
    for c in range(NCORES):
        ci = c % (S // TL)
        q_idx = ci * TL + np.arange(TL)[None, :]
        maskT = np.where(k_idx <= q_idx, 0.0, NEG).astype(f32)
        per_core.append({
            "x": np.ascontiguousarray(x[c * TL:(c + 1) * TL]),
            "maskT": np.ascontiguousarray(maskT),
            **shared,
        })
    return per_core


def kernel(**inputs):
    global _PROG
    if _PROG is None:
        _PROG = build(dbg=False)
    nc, _ = _PROG
    per_core = prep_inputs(inputs)
    res = run_bass_kernel_spmd(nc, per_core, core_ids=list(range(NCORES)))
    y = np.concatenate([res.results[c]["y"] for c in range(NCORES)], axis=0)
    return y.reshape(B, S, D).astype(np.float32)

